# revision 20
# baseline (speedup 1.0000x reference)
"""ConvCNP1d Trainium2 kernel.

Data-parallel over batch: 16 batches -> 8 cores x 2 batches. Each core:
  K1[x,t] = exp(a_psi*(x-t)^2) generated on PE as a rank-3 matmul
    (host-precomputed [a*x'^2, -2a*x', a] x [1, t', t'^2] tables, coords
    centered per t-chunk so the exponent partials stay small wherever the
    kernel value is non-negligible), Exp on the scalar engine,
  h = phi^T @ K1 accumulated on PE (phi = os_psi*[1, yc] packed as weights),
  conv1d stack as 5-tap accumulating matmuls,
  K2[t,xt] generated the same way, mu/sigma = f'^T @ K2 on PE.
The only host work is building the small coefficient tables (O(B*N)).
"""

import numpy as np

T_GRID = 2048
B = 16
N = 2048          # Nc == Nt == 2048
NCORES = 8
BLOC = B // NCORES
EPS = 1e-8

_PROG_CACHE = {}


def build_program():
    import concourse.bacc as bacc
    import concourse.tile as tile
    from concourse import mybir

    f32 = mybir.dt.float32
    AF = mybir.ActivationFunctionType
    # Bacc (not raw Bass): its compile() splits multi-sem waits into event
    # semaphores / ldweights, which the TRN2 ISA requires (1 wait per inst).
    nc = bacc.Bacc(None, target_bir_lowering=False)

    t_in = nc.declare_dram_parameter("t_in", [1, T_GRID], f32, isOutput=False)
    T1h = nc.declare_dram_parameter("T1", [3, T_GRID], f32, isOutput=False)
    T2h = nc.declare_dram_parameter("T2", [3, T_GRID], f32, isOutput=False)
    X1h = nc.declare_dram_parameter("X1", [BLOC, 3, 2, N], f32, isOutput=False)
    X2h = nc.declare_dram_parameter("X2", [BLOC, 3, 4, N], f32, isOutput=False)
    PHIh = nc.declare_dram_parameter("PHI", [BLOC, 128, 32], f32, isOutput=False)
    W1h = nc.declare_dram_parameter("W1", [3, 80], f32, isOutput=False)
    W2h = nc.declare_dram_parameter("W2", [16, 160], f32, isOutput=False)
    W3h = nc.declare_dram_parameter("W3", [32, 80], f32, isOutput=False)
    W4h = nc.declare_dram_parameter("W4", [16, 10], f32, isOutput=False)
    B1h = nc.declare_dram_parameter("B1", [16, 1], f32, isOutput=False)
    B2h = nc.declare_dram_parameter("B2", [32, 1], f32, isOutput=False)
    B3h = nc.declare_dram_parameter("B3", [16, 1], f32, isOutput=False)
    Ch = nc.declare_dram_parameter("CONSTS", [2, 4], f32, isOutput=False)
    ID2h = nc.declare_dram_parameter("ID2", [2, 2], f32, isOutput=False)
    OUTh = nc.declare_dram_parameter("out", [BLOC, 2, T_GRID], f32, isOutput=True)

    with tile.TileContext(nc) as tc:
        with (
            tc.tile_pool(name="singles", bufs=1) as singles,
            tc.tile_pool(name="perb", bufs=2) as perb,
            tc.tile_pool(name="perb1", bufs=1) as perb1,
            tc.tile_pool(name="kpool", bufs=4) as kpool,
            tc.tile_pool(name="small", bufs=1) as small,
            tc.tile_pool(name="outs", bufs=2) as outs,
            tc.tile_pool(name="psd2", bufs=2, space="PSUM") as psd2,
            tc.tile_pool(name="psacc", bufs=2, space="PSUM") as psacc,
        ):
            T1_sb = singles.tile([3, T_GRID], f32)
            nc.sync.dma_start(out=T1_sb, in_=T1h[:, :])
            T2_sb = singles.tile([3, T_GRID], f32)
            nc.sync.dma_start(out=T2_sb, in_=T2h[:, :])
            W1_sb = singles.tile([3, 80], f32)
            nc.sync.dma_start(out=W1_sb, in_=W1h[:, :])
            W2_sb = singles.tile([16, 160], f32)
            nc.sync.dma_start(out=W2_sb, in_=W2h[:, :])
            W3_sb = singles.tile([32, 80], f32)
            nc.sync.dma_start(out=W3_sb, in_=W3h[:, :])
            W4_sb = singles.tile([16, 10], f32)
            nc.sync.dma_start(out=W4_sb, in_=W4h[:, :])
            B1_sb = singles.tile([16, 1], f32)
            nc.sync.dma_start(out=B1_sb, in_=B1h[:, :])
            B2_sb = singles.tile([32, 1], f32)
            nc.sync.dma_start(out=B2_sb, in_=B2h[:, :])
            B3_sb = singles.tile([16, 1], f32)
            nc.sync.dma_start(out=B3_sb, in_=B3h[:, :])
            C_sb = singles.tile([2, 4], f32)
            nc.sync.dma_start(out=C_sb, in_=Ch[:, :])
            ID2_sb = singles.tile([2, 2], f32)
            nc.sync.dma_start(out=ID2_sb, in_=ID2h[:, :])

            for b in range(BLOC):
                X1_sb = perb.tile([3, 2, N], f32, tag="X1")
                nc.sync.dma_start(out=X1_sb, in_=X1h[b])
                X2_sb = perb1.tile([3, 4, N], f32, tag="X2")
                nc.sync.dma_start(out=X2_sb, in_=X2h[b])
                PHI_sb = perb.tile([128, 32], f32, tag="PHI")
                nc.sync.dma_start(out=PHI_sb, in_=PHIh[b])

                rep = perb1.tile([3, T_GRID + 4], f32, tag="rep")
                nc.vector.memset(rep[:, 0:2], 0.0)
                nc.vector.memset(rep[:, T_GRID + 2 : T_GRID + 4], 0.0)
                nc.sync.dma_start(out=rep[0:1, 2 : 2 + T_GRID], in_=t_in[:, :])

                # ---------------- stage A: encoder ----------------
                h_ps = [None, None]
                kq = []

                def gen_enc(s):
                    n2, i = divmod(s, 16)
                    d2 = psd2.tile([128, 1024], f32, tag="d2")
                    K1 = kpool.tile([128, 1024], f32, tag="K")
                    for hh in range(2):
                        nc.tensor.matmul(
                            d2[:, 512 * hh : 512 * (hh + 1)],
                            X1_sb[:, n2, 128 * i : 128 * (i + 1)],
                            T1_sb[:, 1024 * n2 + 512 * hh : 1024 * n2 + 512 * (hh + 1)],
                            start=True,
                            stop=True,
                        )
                    nc.scalar.activation(out=K1, in_=d2, func=AF.Exp)
                    kq.append((K1, n2, i))

                def acc_enc():
                    K1, n2, i = kq.pop(0)
                    if i == 0:
                        h_ps[n2] = psacc.tile([2, 1024], f32, tag="acc", name="h_acc")
                    for hh in range(2):
                        nc.tensor.matmul(
                            h_ps[n2][:, 512 * hh : 512 * (hh + 1)],
                            PHI_sb[:, 2 * i : 2 * i + 2],
                            K1[:, 512 * hh : 512 * (hh + 1)],
                            start=(i == 0),
                            stop=(i == 15),
                        )
                    if i == 15:
                        # engines can only address partition bases 0/32/64,
                        # so single-row math happens in base-0 tiles and DMA
                        # (which has no base restriction) places rep rows 1/2.
                        sl = slice(2 + 1024 * n2, 2 + 1024 * (n2 + 1))
                        h_sb = small.tile([2, 1024], f32, tag="h_sb", name="h_sb")
                        h1_sb = small.tile([1, 1024], f32, tag="h1_sb", name="h1_sb")
                        tmp = small.tile([1, 1024], f32, tag="tmp")
                        rec = small.tile([1, 1024], f32, tag="rec")
                        rat = small.tile([1, 1024], f32, tag="rat")
                        nc.vector.tensor_copy(h_sb, h_ps[n2][:, :])
                        nc.sync.dma_start(out=h1_sb, in_=h_sb[1:2, :])
                        nc.vector.tensor_scalar_add(tmp, h_sb[0:1, :], EPS)
                        nc.vector.reciprocal(rec, tmp)
                        nc.vector.tensor_mul(rat, h1_sb, rec)
                        nc.sync.dma_start(out=rep[1:2, sl], in_=h_sb[0:1, :])
                        nc.sync.dma_start(out=rep[2:3, sl], in_=rat)

                for s in range(33):
                    if s < 32:
                        gen_enc(s)
                    if s >= 1:
                        acc_enc()

                # ---------------- stage B: conv stack ----------------
                def convlayer(in_tile, w_sb, bias_sb, O, out_tile):
                    for n2 in range(2):
                        ps = psacc.tile([O, 1024], f32, tag="acc")
                        for hh in range(2):
                            base = 1024 * n2 + 512 * hh
                            for o in range(5):
                                nc.tensor.matmul(
                                    ps[:, 512 * hh : 512 * (hh + 1)],
                                    w_sb[:, o * O : (o + 1) * O],
                                    in_tile[:, base + o : base + o + 512],
                                    start=(o == 0),
                                    stop=(o == 4),
                                )
                        nc.scalar.activation(
                            out=out_tile[:, 2 + 1024 * n2 : 2 + 1024 * (n2 + 1)],
                            in_=ps,
                            func=AF.Relu,
                            bias=bias_sb,
                        )

                f1 = perb1.tile([16, T_GRID + 4], f32, tag="f1")
                f2 = perb1.tile([32, T_GRID + 4], f32, tag="f2")
                f3 = perb1.tile([16, T_GRID + 4], f32, tag="f3")
                for ft in (f1, f2, f3):
                    nc.vector.memset(ft[:, 0:2], 0.0)
                    nc.vector.memset(ft[:, T_GRID + 2 : T_GRID + 4], 0.0)

                convlayer(rep, W1_sb, B1_sb, 16, f1)
                convlayer(f1, W2_sb, B2_sb, 32, f2)
                convlayer(f2, W3_sb, B3_sb, 16, f3)

                # conv4 -> fpr rows [os*(f+b4_0), os*softplus(f+b4_1)]
                fpr = perb1.tile([2, T_GRID], f32, tag="fpr")
                for n2 in range(2):
                    ps = psacc.tile([2, 1024], f32, tag="acc")
                    for hh in range(2):
                        base = 1024 * n2 + 512 * hh
                        for o in range(5):
                            nc.tensor.matmul(
                                ps[:, 512 * hh : 512 * (hh + 1)],
                                W4_sb[:, o * 2 : (o + 1) * 2],
                                f3[:, base + o : base + o + 512],
                                start=(o == 0),
                                stop=(o == 4),
                            )
                    sl = slice(1024 * n2, 1024 * (n2 + 1))
                    f4_sb = small.tile([2, 1024], f32, tag="f4_sb", name="f4_sb")
                    f4sg = small.tile([1, 1024], f32, tag="f4sg", name="f4sg")
                    fsg = small.tile([1, 1024], f32, tag="fsg", name="fsg")
                    sa = small.tile([1, 1024], f32, tag="sa", name="sa")
                    sr = small.tile([1, 1024], f32, tag="sr", name="sr")
                    nc.vector.tensor_copy(f4_sb, ps[:, :])
                    nc.sync.dma_start(out=f4sg, in_=f4_sb[1:2, :])
                    nc.scalar.activation(
                        out=fpr[0:1, sl],
                        in_=f4_sb[0:1, :],
                        func=AF.Identity,
                        bias=C_sb[0:1, 0:1],
                        scale=C_sb[0:1, 2:3],
                    )
                    # softplus(x+b) = relu(x+b) + ln(1 + exp(-|x+b|));
                    # the compiler has no table set with both Exp and Softplus,
                    # so compose it from set-6 funcs (abs/exp/ln/relu).
                    nc.scalar.activation(
                        out=sa, in_=f4sg, func=AF.Abs, bias=C_sb[0:1, 1:2]
                    )
                    nc.scalar.activation(
                        out=sa, in_=sa, func=AF.Exp, scale=-1.0
                    )
                    nc.scalar.activation(
                        out=sa, in_=sa, func=AF.Ln, bias=1.0
                    )
                    nc.scalar.activation(
                        out=sr, in_=f4sg, func=AF.Relu, bias=C_sb[0:1, 1:2]
                    )
                    nc.vector.tensor_add(fsg, sa, sr)
                    nc.vector.tensor_scalar_mul(fsg, fsg, C_sb[0:1, 2:3])
                    nc.sync.dma_start(out=fpr[1:2, sl], in_=fsg)

                # transpose fpr -> fT[p, c, j] = fpr[c, 128j+p]
                fT = perb1.tile([128, 2, 16], f32, tag="fT")
                for j in range(16):
                    tp = psd2.tile([128, 2], f32, tag="d2")
                    nc.tensor.transpose(
                        tp, fpr[:, 128 * j : 128 * (j + 1)], ID2_sb
                    )
                    nc.vector.tensor_copy(fT[:, :, j], tp)

                # ---------------- stage C: decoder ----------------
                ms_ps = [None, None]
                kq2 = []

                def gen_dec(s):
                    n2, j = divmod(s, 16)
                    d2 = psd2.tile([128, 1024], f32, tag="d2")
                    K2 = kpool.tile([128, 1024], f32, tag="K")
                    for hh in range(2):
                        nc.tensor.matmul(
                            d2[:, 512 * hh : 512 * (hh + 1)],
                            T2_sb[:, 128 * j : 128 * (j + 1)],
                            X2_sb[:, j // 4,
                                  1024 * n2 + 512 * hh : 1024 * n2 + 512 * (hh + 1)],
                            start=True,
                            stop=True,
                        )
                    nc.scalar.activation(out=K2, in_=d2, func=AF.Exp)
                    kq2.append((K2, n2, j))

                def acc_dec():
                    K2, n2, j = kq2.pop(0)
                    if j == 0:
                        ms_ps[n2] = psacc.tile([2, 1024], f32, tag="acc", name="ms_acc")
                    for hh in range(2):
                        nc.tensor.matmul(
                            ms_ps[n2][:, 512 * hh : 512 * (hh + 1)],
                            fT[:, :, j],
                            K2[:, 512 * hh : 512 * (hh + 1)],
                            start=(j == 0),
                            stop=(j == 15),
                        )
                    if j == 15:
                        ms_sb = outs.tile([2, 1024], f32, tag="ms_sb", name="ms_sb")
                        nc.vector.tensor_copy(ms_sb, ms_ps[n2][:, :])
                        nc.sync.dma_start(
                            out=OUTh[b, :, 1024 * n2 : 1024 * (n2 + 1)],
                            in_=ms_sb,
                        )

                for s in range(33):
                    if s < 32:
                        gen_dec(s)
                    if s >= 1:
                        acc_dec()

    nc.compile()
    return nc


def make_inmaps(inputs):
    """Host-side table construction. Returns list of 8 per-core input dicts."""
    f32 = np.float32
    f64 = np.float64
    xc = np.asarray(inputs["xc"])[..., 0].astype(f32)
    yc = np.asarray(inputs["yc"])[..., 0].astype(f32)
    xt = np.asarray(inputs["xt"])[..., 0].astype(f32)
    ls_psi = f64(np.float32(inputs["ls_psi"]))
    os_psi = f64(np.float32(inputs["os_psi"]))
    ls_rho = f64(np.float32(inputs["ls_rho"]))
    os_rho = f64(np.float32(inputs["os_rho"]))
    w = [np.asarray(inputs[f"w{i}"]).astype(f32) for i in (1, 2, 3, 4)]
    bs = [np.asarray(inputs[f"b{i}"]).astype(f32) for i in (1, 2, 3, 4)]

    lower = np.minimum(xc.min(), xt.min())
    upper = np.maximum(xc.max(), xt.max())
    t64 = np.linspace(f64(lower), f64(upper), T_GRID)
    t = t64.astype(f32)

    a_psi = -0.5 / (ls_psi * ls_psi)
    a_rho = -0.5 / (ls_rho * ls_rho)

    cA = np.array([(t64[h * 1024] + t64[h * 1024 + 1023]) / 2 for h in range(2)])
    cB = np.array([(t64[j * 512] + t64[j * 512 + 511]) / 2 for j in range(4)])

    T1 = np.zeros((3, T_GRID), f32)
    T2 = np.zeros((3, T_GRID), f32)
    for h in range(2):
        sl = slice(h * 1024, (h + 1) * 1024)
        tp = t64[sl] - cA[h]
        T1[0, sl] = 1.0
        T1[1, sl] = tp.astype(f32)
        T1[2, sl] = (tp * tp).astype(f32)
    for j in range(4):
        sl = slice(j * 512, (j + 1) * 512)
        tp = t64[sl] - cB[j]
        T2[0, sl] = (a_rho * tp * tp).astype(f32)
        T2[1, sl] = (-2.0 * a_rho * tp).astype(f32)
        T2[2, sl] = a_rho

    X1 = np.zeros((B, 3, 2, N), f32)
    X2 = np.zeros((B, 3, 4, N), f32)
    PHI = np.zeros((B, 128, 32), f32)
    for bi in range(B):
        xcb = xc[bi].astype(f64)
        xtb = xt[bi].astype(f64)
        for h in range(2):
            xp = xcb - cA[h]
            X1[bi, 0, h] = (a_psi * xp * xp).astype(f32)
            X1[bi, 1, h] = (-2.0 * a_psi * xp).astype(f32)
            X1[bi, 2, h] = a_psi
        for j in range(4):
            xp = xtb - cB[j]
            X2[bi, 0, j] = 1.0
            X2[bi, 1, j] = xp.astype(f32)
            X2[bi, 2, j] = (xp * xp).astype(f32)
        phi_full = np.stack([np.full(N, os_psi), os_psi * yc[bi].astype(f64)], 1)
        PHI[bi] = phi_full.astype(f32).reshape(16, 128, 2).transpose(1, 0, 2).reshape(128, 32)

    def pack_w(wl):
        O, I, _ = wl.shape
        out = np.zeros((I, 5 * O), f32)
        for o in range(5):
            out[:, o * O : (o + 1) * O] = wl[:, :, o].T
        return out

    consts = np.zeros((2, 4), f32)
    consts[:, 0] = f32(os_rho * f64(bs[3][0]))
    consts[:, 1] = bs[3][1]
    consts[:, 2] = f32(os_rho)

    shared = {
        "t_in": t[None, :].copy(),
        "T1": T1,
        "T2": T2,
        "W1": pack_w(w[0]),
        "W2": pack_w(w[1]),
        "W3": pack_w(w[2]),
        "W4": pack_w(w[3]),
        "B1": bs[0][:, None].copy(),
        "B2": bs[1][:, None].copy(),
        "B3": bs[2][:, None].copy(),
        "CONSTS": consts,
        "ID2": np.eye(2, dtype=f32),
    }
    in_maps = []
    for c in range(NCORES):
        sl = slice(c * BLOC, (c + 1) * BLOC)
        m = dict(shared)
        m["X1"] = np.ascontiguousarray(X1[sl])
        m["X2"] = np.ascontiguousarray(X2[sl])
        m["PHI"] = np.ascontiguousarray(PHI[sl])
        in_maps.append(m)
    return in_maps


def _get_program():
    if "nc" not in _PROG_CACHE:
        _PROG_CACHE["nc"] = build_program()
    return _PROG_CACHE["nc"]


def kernel(**inputs):
    from concourse.bass_utils import run_bass_kernel_spmd

    nc = _get_program()
    in_maps = make_inmaps(inputs)
    res = run_bass_kernel_spmd(nc, in_maps, core_ids=list(range(NCORES)))
    outs = [np.asarray(res.results[i]["out"]) for i in range(NCORES)]
    full = np.concatenate(outs, 0)  # [B, 2, T]
    return np.ascontiguousarray(full.transpose(0, 2, 1)).astype(np.float32)


# revision 21
# speedup vs baseline: 1.0429x; 1.0429x over previous
"""ConvCNP1d Trainium2 kernel.

Data-parallel over batch: 16 batches -> 8 cores x 2 batches. Each core:
  K1[x,t] = exp(a_psi*(x-t)^2) generated on PE as a rank-3 matmul
    (host-precomputed [a*x'^2, -2a*x', a] x [1, t', t'^2] tables, coords
    centered per t-chunk so the exponent partials stay small wherever the
    kernel value is non-negligible), Exp on the scalar engine,
  h = phi^T @ K1 accumulated on PE (phi = os_psi*[1, yc] packed as weights),
  conv1d stack as 5-tap accumulating matmuls,
  K2[t,xt] generated the same way, mu/sigma = f'^T @ K2 on PE.

Stages are emitted interleaved across the two batches
(A0 A1 B0 B1 T0 C0 T1 C1) so the PE always has matmul work while the
h/f4 epilogue chains (DVE + SBUF-to-SBUF DMA row moves, needed because
compute engines can only address partition bases 0/32/64) drain, keeping
the HAM clock at full rate.
"""

import numpy as np

T_GRID = 2048
B = 16
N = 2048          # Nc == Nt == 2048
NCORES = 8
BLOC = B // NCORES
EPS = 1e-8

_PROG_CACHE = {}


def build_program():
    import concourse.bacc as bacc
    import concourse.tile as tile
    from concourse import mybir

    f32 = mybir.dt.float32
    AF = mybir.ActivationFunctionType
    # Bacc (not raw Bass): its compile() splits multi-sem waits into event
    # semaphores / ldweights, which the TRN2 ISA requires (1 wait per inst).
    nc = bacc.Bacc(None, target_bir_lowering=False)

    t_in = nc.declare_dram_parameter("t_in", [1, T_GRID], f32, isOutput=False)
    T1h = nc.declare_dram_parameter("T1", [3, T_GRID], f32, isOutput=False)
    T2h = nc.declare_dram_parameter("T2", [3, T_GRID], f32, isOutput=False)
    X1h = nc.declare_dram_parameter("X1", [BLOC, 3, 2, N], f32, isOutput=False)
    X2h = nc.declare_dram_parameter("X2", [BLOC, 3, 2, N], f32, isOutput=False)
    PHIh = nc.declare_dram_parameter("PHI", [BLOC, 128, 32], f32, isOutput=False)
    W1h = nc.declare_dram_parameter("W1", [3, 80], f32, isOutput=False)
    W2h = nc.declare_dram_parameter("W2", [16, 160], f32, isOutput=False)
    W3h = nc.declare_dram_parameter("W3", [32, 80], f32, isOutput=False)
    W4h = nc.declare_dram_parameter("W4", [16, 10], f32, isOutput=False)
    B1h = nc.declare_dram_parameter("B1", [16, 1], f32, isOutput=False)
    B2h = nc.declare_dram_parameter("B2", [32, 1], f32, isOutput=False)
    B3h = nc.declare_dram_parameter("B3", [16, 1], f32, isOutput=False)
    Ch = nc.declare_dram_parameter("CONSTS", [2, 4], f32, isOutput=False)
    ID2h = nc.declare_dram_parameter("ID2", [2, 2], f32, isOutput=False)
    OUTh = nc.declare_dram_parameter("out", [BLOC, 2, T_GRID], f32, isOutput=True)

    with tile.TileContext(nc) as tc:
        with (
            tc.tile_pool(name="singles", bufs=1) as singles,
            tc.tile_pool(name="perb", bufs=2) as perb,
            tc.tile_pool(name="perb1", bufs=1) as perb1,
            tc.tile_pool(name="kpool", bufs=3) as kpool,
            tc.tile_pool(name="small", bufs=1) as small,
            tc.tile_pool(name="outs", bufs=2) as outs,
            tc.tile_pool(name="psd2", bufs=4, space="PSUM") as psd2,
            tc.tile_pool(name="psacc", bufs=2, space="PSUM") as psacc,
        ):
            T1_sb = singles.tile([3, T_GRID], f32)
            nc.sync.dma_start(out=T1_sb, in_=T1h[:, :])
            T2_sb = singles.tile([3, T_GRID], f32)
            nc.sync.dma_start(out=T2_sb, in_=T2h[:, :])
            W1_sb = singles.tile([3, 80], f32)
            nc.sync.dma_start(out=W1_sb, in_=W1h[:, :])
            W2_sb = singles.tile([16, 160], f32)
            nc.sync.dma_start(out=W2_sb, in_=W2h[:, :])
            W3_sb = singles.tile([32, 80], f32)
            nc.sync.dma_start(out=W3_sb, in_=W3h[:, :])
            W4_sb = singles.tile([16, 10], f32)
            nc.sync.dma_start(out=W4_sb, in_=W4h[:, :])
            B1_sb = singles.tile([16, 1], f32)
            nc.sync.dma_start(out=B1_sb, in_=B1h[:, :])
            B2_sb = singles.tile([32, 1], f32)
            nc.sync.dma_start(out=B2_sb, in_=B2h[:, :])
            B3_sb = singles.tile([16, 1], f32)
            nc.sync.dma_start(out=B3_sb, in_=B3h[:, :])
            C_sb = singles.tile([2, 4], f32)
            nc.sync.dma_start(out=C_sb, in_=Ch[:, :])
            ID2_sb = singles.tile([2, 2], f32)
            nc.sync.dma_start(out=ID2_sb, in_=ID2h[:, :])

            st = [dict() for _ in range(BLOC)]  # per-batch tile handles

            def loads(b):
                s = st[b]
                s["X1"] = perb.tile([3, 2, N], f32, tag="X1", name="X1_sb")
                nc.sync.dma_start(out=s["X1"], in_=X1h[b])
                s["X2"] = perb.tile([3, 2, N], f32, tag="X2", name="X2_sb")
                nc.sync.dma_start(out=s["X2"], in_=X2h[b])
                s["PHI"] = perb.tile([128, 32], f32, tag="PHI", name="PHI_sb")
                nc.sync.dma_start(out=s["PHI"], in_=PHIh[b])
                rep = perb.tile([3, T_GRID + 4], f32, tag="rep", name="rep")
                nc.vector.memset(rep[:, 0:2], 0.0)
                nc.vector.memset(rep[:, T_GRID + 2 : T_GRID + 4], 0.0)
                nc.sync.dma_start(out=rep[0:1, 2 : 2 + T_GRID], in_=t_in[:, :])
                s["rep"] = rep

            def stage_a(b):
                s = st[b]
                X1_sb, PHI_sb, rep = s["X1"], s["PHI"], s["rep"]
                h_ps = [None, None]
                kq = []

                def gen_enc(sq):
                    n2, i = divmod(sq, 16)
                    K1 = kpool.tile([128, 1024], f32, tag="K", name="K1")
                    for hh in range(2):
                        d2 = psd2.tile([128, 512], f32, tag="d2", name="d2")
                        nc.tensor.matmul(
                            d2,
                            X1_sb[:, n2, 128 * i : 128 * (i + 1)],
                            T1_sb[:, 1024 * n2 + 512 * hh : 1024 * n2 + 512 * (hh + 1)],
                            start=True,
                            stop=True,
                        )
                        nc.scalar.activation(
                            out=K1[:, 512 * hh : 512 * (hh + 1)], in_=d2, func=AF.Exp
                        )
                    kq.append((K1, n2, i))

                def acc_enc():
                    K1, n2, i = kq.pop(0)
                    if i == 0:
                        h_ps[n2] = psacc.tile([2, 1024], f32, tag="acc", name="h_acc")
                    for hh in range(2):
                        nc.tensor.matmul(
                            h_ps[n2][:, 512 * hh : 512 * (hh + 1)],
                            PHI_sb[:, 2 * i : 2 * i + 2],
                            K1[:, 512 * hh : 512 * (hh + 1)],
                            start=(i == 0),
                            stop=(i == 15),
                        )
                    if i == 15:
                        # engines can only address partition bases 0/32/64,
                        # so single-row math happens in base-0 tiles and DMA
                        # (which has no base restriction) places rep rows 1/2.
                        sl = slice(2 + 1024 * n2, 2 + 1024 * (n2 + 1))
                        h_sb = small.tile([2, 1024], f32, tag="h_sb", name="h_sb")
                        h1_sb = small.tile([1, 1024], f32, tag="h1_sb", name="h1_sb")
                        tmp = small.tile([1, 1024], f32, tag="tmp", name="tmp")
                        rec = small.tile([1, 1024], f32, tag="rec", name="rec")
                        rat = small.tile([1, 1024], f32, tag="rat", name="rat")
                        nc.vector.tensor_copy(h_sb, h_ps[n2][:, :])
                        nc.sync.dma_start(out=h1_sb, in_=h_sb[1:2, :])
                        nc.vector.tensor_scalar_add(tmp, h_sb[0:1, :], EPS)
                        nc.vector.reciprocal(rec, tmp)
                        nc.vector.tensor_mul(rat, h1_sb, rec)
                        nc.sync.dma_start(out=rep[1:2, sl], in_=h_sb[0:1, :])
                        nc.sync.dma_start(out=rep[2:3, sl], in_=rat)

                for sq in range(33):
                    if sq < 32:
                        gen_enc(sq)
                    if sq >= 1:
                        acc_enc()

            def stage_b(b):
                s = st[b]
                rep = s["rep"]

                def convlayer(in_tile, w_sb, bias_sb, O, out_tile):
                    for n2 in range(2):
                        ps = psacc.tile([O, 1024], f32, tag="acc", name="conv_ps")
                        for hh in range(2):
                            base = 1024 * n2 + 512 * hh
                            for o in range(5):
                                nc.tensor.matmul(
                                    ps[:, 512 * hh : 512 * (hh + 1)],
                                    w_sb[:, o * O : (o + 1) * O],
                                    in_tile[:, base + o : base + o + 512],
                                    start=(o == 0),
                                    stop=(o == 4),
                                )
                        nc.scalar.activation(
                            out=out_tile[:, 2 + 1024 * n2 : 2 + 1024 * (n2 + 1)],
                            in_=ps,
                            func=AF.Relu,
                            bias=bias_sb,
                        )

                f1 = perb1.tile([16, T_GRID + 4], f32, tag="f1", name="f1")
                f2 = perb1.tile([32, T_GRID + 4], f32, tag="f2", name="f2")
                f3 = perb1.tile([16, T_GRID + 4], f32, tag="f3", name="f3")
                for ft in (f1, f2, f3):
                    nc.vector.memset(ft[:, 0:2], 0.0)
                    nc.vector.memset(ft[:, T_GRID + 2 : T_GRID + 4], 0.0)

                convlayer(rep, W1_sb, B1_sb, 16, f1)
                convlayer(f1, W2_sb, B2_sb, 32, f2)
                convlayer(f2, W3_sb, B3_sb, 16, f3)

                # conv4 -> fpr rows [os*(f+b4_0), os*softplus(f+b4_1)]
                fpr = perb.tile([2, T_GRID], f32, tag="fpr", name="fpr")
                s["fpr"] = fpr
                for n2 in range(2):
                    ps = psacc.tile([2, 1024], f32, tag="acc", name="c4_ps")
                    for hh in range(2):
                        base = 1024 * n2 + 512 * hh
                        for o in range(5):
                            nc.tensor.matmul(
                                ps[:, 512 * hh : 512 * (hh + 1)],
                                W4_sb[:, o * 2 : (o + 1) * 2],
                                f3[:, base + o : base + o + 512],
                                start=(o == 0),
                                stop=(o == 4),
                            )
                    sl = slice(1024 * n2, 1024 * (n2 + 1))
                    f4_sb = small.tile([2, 1024], f32, tag="f4_sb", name="f4_sb")
                    f4sg = small.tile([1, 1024], f32, tag="f4sg", name="f4sg")
                    fmu = small.tile([1, 1024], f32, tag="fmu", name="fmu")
                    fsg = small.tile([1, 1024], f32, tag="fsg", name="fsg")
                    sa = small.tile([1, 1024], f32, tag="sa", name="sa")
                    sr = small.tile([1, 1024], f32, tag="sr", name="sr")
                    nc.vector.tensor_copy(f4_sb, ps[:, :])
                    nc.sync.dma_start(out=f4sg, in_=f4_sb[1:2, :])
                    nc.scalar.activation(
                        out=fmu,
                        in_=f4_sb[0:1, :],
                        func=AF.Identity,
                        bias=C_sb[0:1, 0:1],
                        scale=C_sb[0:1, 2:3],
                    )
                    nc.sync.dma_start(out=fpr[0:1, sl], in_=fmu)
                    # softplus(x+b) = relu(x+b) + ln(1 + exp(-|x+b|));
                    # no act table set has both Exp and Softplus, so compose
                    # from set-6 funcs (abs/exp/ln/relu).
                    nc.scalar.activation(
                        out=sa, in_=f4sg, func=AF.Abs, bias=C_sb[0:1, 1:2]
                    )
                    nc.scalar.activation(out=sa, in_=sa, func=AF.Exp, scale=-1.0)
                    nc.scalar.activation(out=sa, in_=sa, func=AF.Ln, bias=1.0)
                    nc.scalar.activation(
                        out=sr, in_=f4sg, func=AF.Relu, bias=C_sb[0:1, 1:2]
                    )
                    nc.vector.tensor_add(fsg, sa, sr)
                    nc.vector.tensor_scalar_mul(fsg, fsg, C_sb[0:1, 2:3])
                    nc.sync.dma_start(out=fpr[1:2, sl], in_=fsg)

            def stage_t(b):
                # transpose fpr -> fT[p, c, j] = fpr[c, 128j+p]
                s = st[b]
                fpr = s["fpr"]
                fT = perb.tile([128, 2, 16], f32, tag="fT", name="fT")
                s["fT"] = fT
                for j in range(16):
                    tp = psd2.tile([128, 2], f32, tag="d2", name="tp")
                    nc.tensor.transpose(
                        tp, fpr[:, 128 * j : 128 * (j + 1)], ID2_sb
                    )
                    nc.scalar.copy(fT[:, :, j], tp)

            def stage_c(b):
                s = st[b]
                X2_sb, fT = s["X2"], s["fT"]
                ms_ps = [None, None]
                kq2 = []

                def gen_dec(sq):
                    n2, j = divmod(sq, 16)
                    K2 = kpool.tile([128, 1024], f32, tag="K", name="K2")
                    for hh in range(2):
                        d2 = psd2.tile([128, 512], f32, tag="d2", name="d2c")
                        nc.tensor.matmul(
                            d2,
                            T2_sb[:, 128 * j : 128 * (j + 1)],
                            X2_sb[:, j // 8,
                                  1024 * n2 + 512 * hh : 1024 * n2 + 512 * (hh + 1)],
                            start=True,
                            stop=True,
                        )
                        nc.scalar.activation(
                            out=K2[:, 512 * hh : 512 * (hh + 1)], in_=d2, func=AF.Exp
                        )
                    kq2.append((K2, n2, j))

                def acc_dec():
                    K2, n2, j = kq2.pop(0)
                    if j == 0:
                        ms_ps[n2] = psacc.tile([2, 1024], f32, tag="acc", name="ms_acc")
                    for hh in range(2):
                        nc.tensor.matmul(
                            ms_ps[n2][:, 512 * hh : 512 * (hh + 1)],
                            fT[:, :, j],
                            K2[:, 512 * hh : 512 * (hh + 1)],
                            start=(j == 0),
                            stop=(j == 15),
                        )
                    if j == 15:
                        ms_sb = outs.tile([2, 1024], f32, tag="ms_sb", name="ms_sb")
                        nc.vector.tensor_copy(ms_sb, ms_ps[n2][:, :])
                        nc.sync.dma_start(
                            out=OUTh[b, :, 1024 * n2 : 1024 * (n2 + 1)],
                            in_=ms_sb,
                        )

                for sq in range(33):
                    if sq < 32:
                        gen_dec(sq)
                    if sq >= 1:
                        acc_dec()

            loads(0)
            loads(1)
            stage_a(0)
            stage_a(1)
            stage_b(0)
            stage_b(1)
            stage_t(0)
            stage_c(0)
            stage_t(1)
            stage_c(1)

    nc.compile()
    return nc


def make_inmaps(inputs):
    """Host-side table construction. Returns list of 8 per-core input dicts."""
    f32 = np.float32
    f64 = np.float64
    xc = np.asarray(inputs["xc"])[..., 0].astype(f32)
    yc = np.asarray(inputs["yc"])[..., 0].astype(f32)
    xt = np.asarray(inputs["xt"])[..., 0].astype(f32)
    ls_psi = f64(np.float32(inputs["ls_psi"]))
    os_psi = f64(np.float32(inputs["os_psi"]))
    ls_rho = f64(np.float32(inputs["ls_rho"]))
    os_rho = f64(np.float32(inputs["os_rho"]))
    w = [np.asarray(inputs[f"w{i}"]).astype(f32) for i in (1, 2, 3, 4)]
    bs = [np.asarray(inputs[f"b{i}"]).astype(f32) for i in (1, 2, 3, 4)]

    lower = np.minimum(xc.min(), xt.min())
    upper = np.maximum(xc.max(), xt.max())
    t64 = np.linspace(f64(lower), f64(upper), T_GRID)
    t = t64.astype(f32)

    a_psi = -0.5 / (ls_psi * ls_psi)
    a_rho = -0.5 / (ls_rho * ls_rho)

    cA = np.array([(t64[h * 1024] + t64[h * 1024 + 1023]) / 2 for h in range(2)])
    cB = np.array([(t64[j * 1024] + t64[j * 1024 + 1023]) / 2 for j in range(2)])

    T1 = np.zeros((3, T_GRID), f32)
    T2 = np.zeros((3, T_GRID), f32)
    for h in range(2):
        sl = slice(h * 1024, (h + 1) * 1024)
        tp = t64[sl] - cA[h]
        T1[0, sl] = 1.0
        T1[1, sl] = tp.astype(f32)
        T1[2, sl] = (tp * tp).astype(f32)
    for j in range(2):
        sl = slice(j * 1024, (j + 1) * 1024)
        tp = t64[sl] - cB[j]
        T2[0, sl] = (a_rho * tp * tp).astype(f32)
        T2[1, sl] = (-2.0 * a_rho * tp).astype(f32)
        T2[2, sl] = a_rho

    X1 = np.zeros((B, 3, 2, N), f32)
    X2 = np.zeros((B, 3, 2, N), f32)
    PHI = np.zeros((B, 128, 32), f32)
    for bi in range(B):
        xcb = xc[bi].astype(f64)
        xtb = xt[bi].astype(f64)
        for h in range(2):
            xp = xcb - cA[h]
            X1[bi, 0, h] = (a_psi * xp * xp).astype(f32)
            X1[bi, 1, h] = (-2.0 * a_psi * xp).astype(f32)
            X1[bi, 2, h] = a_psi
        for j in range(2):
            xp = xtb - cB[j]
            X2[bi, 0, j] = 1.0
            X2[bi, 1, j] = xp.astype(f32)
            X2[bi, 2, j] = (xp * xp).astype(f32)
        phi_full = np.stack([np.full(N, os_psi), os_psi * yc[bi].astype(f64)], 1)
        PHI[bi] = phi_full.astype(f32).reshape(16, 128, 2).transpose(1, 0, 2).reshape(128, 32)

    def pack_w(wl):
        O, I, _ = wl.shape
        out = np.zeros((I, 5 * O), f32)
        for o in range(5):
            out[:, o * O : (o + 1) * O] = wl[:, :, o].T
        return out

    consts = np.zeros((2, 4), f32)
    consts[:, 0] = f32(os_rho * f64(bs[3][0]))
    consts[:, 1] = bs[3][1]
    consts[:, 2] = f32(os_rho)

    shared = {
        "t_in": t[None, :].copy(),
        "T1": T1,
        "T2": T2,
        "W1": pack_w(w[0]),
        "W2": pack_w(w[1]),
        "W3": pack_w(w[2]),
        "W4": pack_w(w[3]),
        "B1": bs[0][:, None].copy(),
        "B2": bs[1][:, None].copy(),
        "B3": bs[2][:, None].copy(),
        "CONSTS": consts,
        "ID2": np.eye(2, dtype=f32),
    }
    in_maps = []
    for c in range(NCORES):
        sl = slice(c * BLOC, (c + 1) * BLOC)
        m = dict(shared)
        m["X1"] = np.ascontiguousarray(X1[sl])
        m["X2"] = np.ascontiguousarray(X2[sl])
        m["PHI"] = np.ascontiguousarray(PHI[sl])
        in_maps.append(m)
    return in_maps


def _get_program():
    if "nc" not in _PROG_CACHE:
        _PROG_CACHE["nc"] = build_program()
    return _PROG_CACHE["nc"]


def kernel(**inputs):
    from concourse.bass_utils import run_bass_kernel_spmd

    nc = _get_program()
    in_maps = make_inmaps(inputs)
    res = run_bass_kernel_spmd(nc, in_maps, core_ids=list(range(NCORES)))
    outs = [np.asarray(res.results[i]["out"]) for i in range(NCORES)]
    full = np.concatenate(outs, 0)  # [B, 2, T]
    return np.ascontiguousarray(full.transpose(0, 2, 1)).astype(np.float32)


# revision 29
# speedup vs baseline: 1.1412x; 1.0942x over previous
"""ConvCNP1d Trainium2 kernel.

Data-parallel over batch: 16 batches -> 8 cores x 2 batches. Each core:
  K1[x,t] = exp(a_psi*(x-t)^2) generated on PE as a rank-3 matmul
    (host-precomputed [a*x'^2, -2a*x', a] x [1, t', t'^2] tables, coords
    centered per t-chunk so the exponent partials stay small wherever the
    kernel value is non-negligible), Exp on the scalar engine,
  h = phi^T @ K1 accumulated on PE (phi = os_psi*[1, yc] packed as weights),
  conv1d stack as 5-tap accumulating matmuls,
  K2[t,xt] generated the same way, mu/sigma = f'^T @ K2 on PE.

Stages are emitted interleaved across the two batches
(A0 A1 B0 B1 T0 C0 T1 C1) so the PE always has matmul work while the
h/f4 epilogue chains (DVE + SBUF-to-SBUF DMA row moves, needed because
compute engines can only address partition bases 0/32/64) drain, keeping
the HAM clock at full rate.
"""

import numpy as np

T_GRID = 2048
B = 16
N = 2048          # Nc == Nt == 2048
NCORES = 8
BLOC = B // NCORES
EPS = 1e-8

_PROG_CACHE = {}


def build_program():
    import concourse.bacc as bacc
    import concourse.tile as tile
    from concourse import mybir

    f32 = mybir.dt.float32
    f16 = mybir.dt.float16
    AF = mybir.ActivationFunctionType
    # Bacc (not raw Bass): its compile() splits multi-sem waits into event
    # semaphores / ldweights, which the TRN2 ISA requires (1 wait per inst).
    nc = bacc.Bacc(None, target_bir_lowering=False)

    t_in = nc.declare_dram_parameter("t_in", [1, T_GRID], f32, isOutput=False)
    T1h = nc.declare_dram_parameter("T1", [3, T_GRID], f32, isOutput=False)
    T2h = nc.declare_dram_parameter("T2", [3, T_GRID], f32, isOutput=False)
    X1h = nc.declare_dram_parameter("X1", [BLOC, 3, 2, N], f32, isOutput=False)
    X2h = nc.declare_dram_parameter("X2", [BLOC, 3, 2, N], f32, isOutput=False)
    PHIh = nc.declare_dram_parameter("PHI", [BLOC, 128, 32], f16, isOutput=False)
    W1h = nc.declare_dram_parameter("W1", [15, 16], f32, isOutput=False)
    W2h = nc.declare_dram_parameter("W2", [80, 32], f32, isOutput=False)
    W3ah = nc.declare_dram_parameter("W3a", [128, 16], f32, isOutput=False)
    W3bh = nc.declare_dram_parameter("W3b", [32, 16], f32, isOutput=False)
    W4h = nc.declare_dram_parameter("W4", [80, 2], f32, isOutput=False)
    B1h = nc.declare_dram_parameter("B1", [16, 1], f32, isOutput=False)
    B2h = nc.declare_dram_parameter("B2", [32, 1], f32, isOutput=False)
    B3h = nc.declare_dram_parameter("B3", [16, 1], f32, isOutput=False)
    Ch = nc.declare_dram_parameter("CONSTS", [2, 4], f32, isOutput=False)
    ID2h = nc.declare_dram_parameter("ID2", [2, 2], f16, isOutput=False)
    OUTh = nc.declare_dram_parameter("out", [BLOC, 2, T_GRID], f32, isOutput=True)

    with tile.TileContext(nc) as tc:
        with (
            tc.tile_pool(name="singles", bufs=1) as singles,
            tc.tile_pool(name="perb", bufs=2) as perb,
            tc.tile_pool(name="perb1", bufs=1) as perb1,
            tc.tile_pool(name="kpool", bufs=3) as kpool,
            tc.tile_pool(name="small", bufs=1) as small,
            tc.tile_pool(name="impool", bufs=4) as impool,
            tc.tile_pool(name="outs", bufs=2) as outs,
            tc.tile_pool(name="psd2", bufs=2, space="PSUM") as psd2,
            tc.tile_pool(name="psacc", bufs=2, space="PSUM") as psacc,
        ):
            T1_sb = singles.tile([3, T_GRID], f32)
            nc.sync.dma_start(out=T1_sb, in_=T1h[:, :])
            T2_sb = singles.tile([3, T_GRID], f32)
            nc.sync.dma_start(out=T2_sb, in_=T2h[:, :])
            W1_sb = singles.tile([15, 16], f32)
            nc.sync.dma_start(out=W1_sb, in_=W1h[:, :])
            W2_sb = singles.tile([80, 32], f32)
            nc.sync.dma_start(out=W2_sb, in_=W2h[:, :])
            W3a_sb = singles.tile([128, 16], f32)
            nc.sync.dma_start(out=W3a_sb, in_=W3ah[:, :])
            W3b_sb = singles.tile([32, 16], f32)
            nc.sync.dma_start(out=W3b_sb, in_=W3bh[:, :])
            W4_sb = singles.tile([80, 2], f32)
            nc.sync.dma_start(out=W4_sb, in_=W4h[:, :])
            B1_sb = singles.tile([16, 1], f32)
            nc.sync.dma_start(out=B1_sb, in_=B1h[:, :])
            B2_sb = singles.tile([32, 1], f32)
            nc.sync.dma_start(out=B2_sb, in_=B2h[:, :])
            B3_sb = singles.tile([16, 1], f32)
            nc.sync.dma_start(out=B3_sb, in_=B3h[:, :])
            C_sb = singles.tile([2, 4], f32)
            nc.sync.dma_start(out=C_sb, in_=Ch[:, :])
            ID2_sb = singles.tile([2, 2], f16)
            nc.sync.dma_start(out=ID2_sb, in_=ID2h[:, :])

            st = [dict() for _ in range(BLOC)]  # per-batch tile handles

            def loads(b):
                s = st[b]
                s["X1"] = perb.tile([3, 2, N], f32, tag="X1", name="X1_sb")
                nc.sync.dma_start(out=s["X1"], in_=X1h[b])
                s["X2"] = perb.tile([3, 2, N], f32, tag="X2", name="X2_sb")
                nc.sync.dma_start(out=s["X2"], in_=X2h[b])
                s["PHI"] = perb.tile([128, 32], f16, tag="PHI", name="PHI_sb")
                nc.sync.dma_start(out=s["PHI"], in_=PHIh[b])
                rep = perb.tile([3, T_GRID + 4], f32, tag="rep", name="rep")
                nc.vector.memset(rep[:, 0:2], 0.0)
                nc.vector.memset(rep[:, T_GRID + 2 : T_GRID + 4], 0.0)
                nc.sync.dma_start(out=rep[0:1, 2 : 2 + T_GRID], in_=t_in[:, :])
                s["rep"] = rep

            def stage_a(b):
                s = st[b]
                X1_sb, PHI_sb, rep = s["X1"], s["PHI"], s["rep"]
                h_ps = [None, None]
                kq = []

                def gen_enc(sq):
                    n2, i = divmod(sq, 16)
                    K1 = kpool.tile([128, 1024], f16, tag="K", name="K1")
                    d2 = psd2.tile([128, 1024], f32, tag="d2", name="d2")
                    for hh in range(2):
                        nc.tensor.matmul(
                            d2[:, 512 * hh : 512 * (hh + 1)],
                            X1_sb[:, n2, 128 * i : 128 * (i + 1)],
                            T1_sb[:, 1024 * n2 + 512 * hh : 1024 * n2 + 512 * (hh + 1)],
                            start=True,
                            stop=True,
                        )
                    nc.scalar.activation(out=K1, in_=d2, func=AF.Exp)
                    kq.append((K1, n2, i))

                def acc_enc():
                    K1, n2, i = kq.pop(0)
                    if i == 0:
                        h_ps[n2] = psacc.tile([2, 1024], f32, tag="acc", name="h_acc")
                    for hh in range(2):
                        nc.tensor.matmul(
                            h_ps[n2][:, 512 * hh : 512 * (hh + 1)],
                            PHI_sb[:, 2 * i : 2 * i + 2],
                            K1[:, 512 * hh : 512 * (hh + 1)],
                            start=(i == 0),
                            stop=(i == 15),
                        )
                    if i == 15:
                        # engines can only address partition bases 0/32/64,
                        # so single-row math happens in base-0 tiles and DMA
                        # (which has no base restriction) places rep rows 1/2.
                        sl = slice(2 + 1024 * n2, 2 + 1024 * (n2 + 1))
                        h_sb = small.tile([2, 1024], f32, tag="h_sb", name="h_sb")
                        h1_sb = small.tile([1, 1024], f32, tag="h1_sb", name="h1_sb")
                        tmp = small.tile([1, 1024], f32, tag="tmp", name="tmp")
                        rec = small.tile([1, 1024], f32, tag="rec", name="rec")
                        rat = small.tile([1, 1024], f32, tag="rat", name="rat")
                        nc.vector.tensor_copy(h_sb, h_ps[n2][:, :])
                        nc.sync.dma_start(out=h1_sb, in_=h_sb[1:2, :])
                        nc.vector.tensor_scalar_add(tmp, h_sb[0:1, :], EPS)
                        nc.vector.reciprocal(rec, tmp)
                        nc.vector.tensor_mul(rat, h1_sb, rec)
                        nc.sync.dma_start(out=rep[1:2, sl], in_=h_sb[0:1, :])
                        nc.sync.dma_start(out=rep[2:3, sl], in_=rat)

                for sq in range(33):
                    if sq < 32:
                        gen_enc(sq)
                    if sq >= 1:
                        acc_enc()

            def stage_b_layer(b, l):
                """conv layer l for batch b, as im2col matmuls over 512-wide
                chunks. 5 shifted row-block DMAs build each im tile; a single
                (or two, for K=160) fp32 matmul contracts taps*channels."""
                s = st[b]
                if l == 0:
                    for nmt, shp in (("f1", 16), ("f2", 32), ("f3", 16)):
                        s[nmt] = perb.tile([shp, T_GRID + 4], f32, tag=nmt, name=nmt)
                        nc.vector.memset(s[nmt][:, 0:2], 0.0)
                        nc.vector.memset(s[nmt][:, T_GRID + 2 : T_GRID + 4], 0.0)
                    s["fpr"] = perb.tile([2, T_GRID], f16, tag="fpr", name="fpr")
                cfg = [
                    # in_tile, I, parts [(w_sb, taps)], O, bias, out
                    (s["rep"], 3, [(W1_sb, (0, 1, 2, 3, 4))], 16, B1_sb, s["f1"]),
                    (s["f1"], 16, [(W2_sb, (0, 1, 2, 3, 4))], 32, B2_sb, s["f2"]),
                    (s["f2"], 32, [(W3a_sb, (0, 1, 2, 3)), (W3b_sb, (4,))], 16,
                     B3_sb, s["f3"]),
                    (s["f3"], 16, [(W4_sb, (0, 1, 2, 3, 4))], 2, None, s["fpr"]),
                ][l]
                in_tile, I, parts, O, bias_sb, out_tile = cfg
                for n in range(4):
                    ims = []
                    for pi, (w_sb, taps) in enumerate(parts):
                        Kp = I * len(taps)
                        im = impool.tile([Kp, 512], f32, tag="im", name=f"im{l}_{pi}")
                        for idx, o in enumerate(taps):
                            nc.sync.dma_start(
                                out=im[idx * I : (idx + 1) * I, :],
                                in_=in_tile[:, 512 * n + o : 512 * n + o + 512],
                            )
                        ims.append((w_sb, im))
                    ps = psacc.tile([O, 512], f32, tag="acc", name="cps")
                    for k, (w_sb, im) in enumerate(ims):
                        nc.tensor.matmul(
                            ps, w_sb, im, start=(k == 0), stop=(k == len(ims) - 1)
                        )
                    if l < 3:
                        nc.scalar.activation(
                            out=out_tile[:, 2 + 512 * n : 2 + 512 * (n + 1)],
                            in_=ps,
                            func=AF.Relu,
                            bias=bias_sb,
                        )
                    else:
                        # conv4 epilogue: fpr rows [os*(f+b4_0), os*softplus(f+b4_1)]
                        sl = slice(512 * n, 512 * (n + 1))
                        f4_sb = small.tile([2, 512], f32, tag="h_sb", name="f4_sb")
                        f4sg = small.tile([1, 512], f32, tag="h1_sb", name="f4sg")
                        fmu = small.tile([1, 512], f16, tag="tmp", name="fmu")
                        fsg = small.tile([1, 512], f16, tag="rec", name="fsg")
                        sa = small.tile([1, 512], f32, tag="rat", name="sa")
                        sr = small.tile([1, 512], f32, tag="sr", name="sr")
                        nc.vector.tensor_copy(f4_sb, ps[:, :])
                        nc.sync.dma_start(out=f4sg, in_=f4_sb[1:2, :])
                        nc.scalar.activation(
                            out=fmu,
                            in_=f4_sb[0:1, :],
                            func=AF.Identity,
                            bias=C_sb[0:1, 0:1],
                            scale=C_sb[0:1, 2:3],
                        )
                        nc.sync.dma_start(out=out_tile[0:1, sl], in_=fmu)
                        # softplus(x+b) = relu(x+b) + ln(1 + exp(-|x+b|));
                        # no act table set has both Exp and Softplus, so compose
                        # from set-6 funcs (abs/exp/ln/relu).
                        nc.scalar.activation(
                            out=sa, in_=f4sg, func=AF.Abs, bias=C_sb[0:1, 1:2]
                        )
                        nc.scalar.activation(out=sa, in_=sa, func=AF.Exp, scale=-1.0)
                        nc.scalar.activation(out=sa, in_=sa, func=AF.Ln, bias=1.0)
                        nc.scalar.activation(
                            out=sr, in_=f4sg, func=AF.Relu, bias=C_sb[0:1, 1:2]
                        )
                        nc.vector.tensor_add(fsg, sa, sr)
                        nc.vector.tensor_scalar_mul(fsg, fsg, C_sb[0:1, 2:3])
                        nc.sync.dma_start(out=out_tile[1:2, sl], in_=fsg)

            def stage_t(b):
                # transpose fpr -> fT[p, c, j] = fpr[c, 128j+p]
                s = st[b]
                fpr = s["fpr"]
                fT = perb.tile([128, 2, 16], f16, tag="fT", name="fT")
                s["fT"] = fT
                for j in range(16):
                    tp = psd2.tile([128, 2], f16, tag="d2", name="tp")
                    nc.tensor.transpose(
                        tp, fpr[:, 128 * j : 128 * (j + 1)], ID2_sb
                    )
                    nc.scalar.copy(fT[:, :, j], tp)

            def stage_c(b):
                s = st[b]
                X2_sb, fT = s["X2"], s["fT"]
                ms_ps = [None, None]
                kq2 = []

                def gen_dec(sq):
                    n2, j = divmod(sq, 16)
                    K2 = kpool.tile([128, 1024], f16, tag="K", name="K2")
                    d2 = psd2.tile([128, 1024], f32, tag="d2", name="d2c")
                    for hh in range(2):
                        nc.tensor.matmul(
                            d2[:, 512 * hh : 512 * (hh + 1)],
                            T2_sb[:, 128 * j : 128 * (j + 1)],
                            X2_sb[:, j // 8,
                                  1024 * n2 + 512 * hh : 1024 * n2 + 512 * (hh + 1)],
                            start=True,
                            stop=True,
                        )
                    nc.scalar.activation(out=K2, in_=d2, func=AF.Exp)
                    kq2.append((K2, n2, j))

                def acc_dec():
                    K2, n2, j = kq2.pop(0)
                    if j == 0:
                        ms_ps[n2] = psacc.tile([2, 1024], f32, tag="acc", name="ms_acc")
                    for hh in range(2):
                        nc.tensor.matmul(
                            ms_ps[n2][:, 512 * hh : 512 * (hh + 1)],
                            fT[:, :, j],
                            K2[:, 512 * hh : 512 * (hh + 1)],
                            start=(j == 0),
                            stop=(j == 15),
                        )
                    if j == 15:
                        ms_sb = outs.tile([2, 1024], f32, tag="ms_sb", name="ms_sb")
                        nc.vector.tensor_copy(ms_sb, ms_ps[n2][:, :])
                        nc.sync.dma_start(
                            out=OUTh[b, :, 1024 * n2 : 1024 * (n2 + 1)],
                            in_=ms_sb,
                        )

                for sq in range(33):
                    if sq < 32:
                        gen_dec(sq)
                    if sq >= 1:
                        acc_dec()

            loads(0)
            loads(1)
            stage_a(0)
            stage_a(1)
            for l in range(4):
                for b in range(BLOC):
                    stage_b_layer(b, l)
            stage_t(0)
            stage_c(0)
            stage_t(1)
            stage_c(1)

    nc.compile()
    return nc


def make_inmaps(inputs):
    """Host-side table construction. Returns list of 8 per-core input dicts."""
    f32 = np.float32
    f64 = np.float64
    xc = np.asarray(inputs["xc"])[..., 0].astype(f32)
    yc = np.asarray(inputs["yc"])[..., 0].astype(f32)
    xt = np.asarray(inputs["xt"])[..., 0].astype(f32)
    ls_psi = f64(np.float32(inputs["ls_psi"]))
    os_psi = f64(np.float32(inputs["os_psi"]))
    ls_rho = f64(np.float32(inputs["ls_rho"]))
    os_rho = f64(np.float32(inputs["os_rho"]))
    w = [np.asarray(inputs[f"w{i}"]).astype(f32) for i in (1, 2, 3, 4)]
    bs = [np.asarray(inputs[f"b{i}"]).astype(f32) for i in (1, 2, 3, 4)]

    lower = np.minimum(xc.min(), xt.min())
    upper = np.maximum(xc.max(), xt.max())
    t64 = np.linspace(f64(lower), f64(upper), T_GRID)
    t = t64.astype(f32)

    a_psi = -0.5 / (ls_psi * ls_psi)
    a_rho = -0.5 / (ls_rho * ls_rho)

    cA = np.array([(t64[h * 1024] + t64[h * 1024 + 1023]) / 2 for h in range(2)])
    cB = np.array([(t64[j * 1024] + t64[j * 1024 + 1023]) / 2 for j in range(2)])

    T1 = np.zeros((3, T_GRID), f32)
    T2 = np.zeros((3, T_GRID), f32)
    for h in range(2):
        sl = slice(h * 1024, (h + 1) * 1024)
        tp = t64[sl] - cA[h]
        T1[0, sl] = 1.0
        T1[1, sl] = tp.astype(f32)
        T1[2, sl] = (tp * tp).astype(f32)
    for j in range(2):
        sl = slice(j * 1024, (j + 1) * 1024)
        tp = t64[sl] - cB[j]
        T2[0, sl] = (a_rho * tp * tp).astype(f32)
        T2[1, sl] = (-2.0 * a_rho * tp).astype(f32)
        T2[2, sl] = a_rho

    X1 = np.zeros((B, 3, 2, N), f32)
    X2 = np.zeros((B, 3, 2, N), f32)
    PHI = np.zeros((B, 128, 32), f32)
    for bi in range(B):
        xcb = xc[bi].astype(f64)
        xtb = xt[bi].astype(f64)
        for h in range(2):
            xp = xcb - cA[h]
            X1[bi, 0, h] = (a_psi * xp * xp).astype(f32)
            X1[bi, 1, h] = (-2.0 * a_psi * xp).astype(f32)
            X1[bi, 2, h] = a_psi
        for j in range(2):
            xp = xtb - cB[j]
            X2[bi, 0, j] = 1.0
            X2[bi, 1, j] = xp.astype(f32)
            X2[bi, 2, j] = (xp * xp).astype(f32)
        phi_full = np.stack([np.full(N, os_psi), os_psi * yc[bi].astype(f64)], 1)
        PHI[bi] = phi_full.astype(f32).reshape(16, 128, 2).transpose(1, 0, 2).reshape(128, 32)

    def pack_im2col(wl, taps):
        # rows idx*I + c = wl[:, c, taps[idx]]^T -> [len(taps)*I, O]
        O, I, _ = wl.shape
        return np.concatenate([wl[:, :, o].T for o in taps], 0).astype(f32)

    consts = np.zeros((2, 4), f32)
    consts[:, 0] = f32(os_rho * f64(bs[3][0]))
    consts[:, 1] = bs[3][1]
    consts[:, 2] = f32(os_rho)

    shared = {
        "t_in": t[None, :].copy(),
        "T1": T1,
        "T2": T2,
        "W1": pack_im2col(w[0], (0, 1, 2, 3, 4)),
        "W2": pack_im2col(w[1], (0, 1, 2, 3, 4)),
        "W3a": pack_im2col(w[2], (0, 1, 2, 3)),
        "W3b": pack_im2col(w[2], (4,)),
        "W4": pack_im2col(w[3], (0, 1, 2, 3, 4)),
        "B1": bs[0][:, None].copy(),
        "B2": bs[1][:, None].copy(),
        "B3": bs[2][:, None].copy(),
        "CONSTS": consts,
        "ID2": np.eye(2, dtype=np.float16),
    }
    in_maps = []
    for c in range(NCORES):
        sl = slice(c * BLOC, (c + 1) * BLOC)
        m = dict(shared)
        m["X1"] = np.ascontiguousarray(X1[sl])
        m["X2"] = np.ascontiguousarray(X2[sl])
        m["PHI"] = np.ascontiguousarray(PHI[sl].astype(np.float16))
        in_maps.append(m)
    return in_maps


def _get_program():
    if "nc" not in _PROG_CACHE:
        _PROG_CACHE["nc"] = build_program()
    return _PROG_CACHE["nc"]


def kernel(**inputs):
    from concourse.bass_utils import run_bass_kernel_spmd

    nc = _get_program()
    in_maps = make_inmaps(inputs)
    res = run_bass_kernel_spmd(nc, in_maps, core_ids=list(range(NCORES)))
    outs = [np.asarray(res.results[i]["out"]) for i in range(NCORES)]
    full = np.concatenate(outs, 0)  # [B, 2, T]
    return np.ascontiguousarray(full.transpose(0, 2, 1)).astype(np.float32)


# revision 30
# speedup vs baseline: 1.1479x; 1.0059x over previous
"""ConvCNP1d Trainium2 kernel.

Data-parallel over batch: 16 batches -> 8 cores x 2 batches. Each core:
  K1[x,t] = exp(a_psi*(x-t)^2) generated on PE as a rank-3 fp32 matmul
    (host-precomputed [a*x'^2, -2a*x', a] x [1, t', t'^2] tables, coords
    centered per t-chunk so the exponent partials stay small wherever the
    kernel value is non-negligible), Exp on the scalar engine into fp16,
  h = phi^T @ K1 accumulated on PE in fp16 (phi = os_psi*[1, yc] weights),
  conv1d stack as 5-tap fp16 accumulating matmuls; conv1's t-row
    contribution (linear, data-independent of the device) is precomputed
    on the host in fp64 and added via DVE, so only the h0 / h1-ratio rows
    run on-device; conv4 accumulates mu and sigma rows separately so both
    land at partition base 0 (compute engines can only address partition
    bases 0/32/64),
  K2[t,xt] generated the same way, mu/sigma = f'^T @ K2 in fp16.

Stages are emitted interleaved across the two batches
(A0 A1 B0 B1 T0 C0 T1 C1) so the PE always has matmul work while the
h-epilogue chains (DVE + two SBUF-to-SBUF DMA row moves) drain.
SBUF-to-SBUF DMAs have ~15us latency on this path, so the kernel uses
them only in the h epilogue, where stage-A work of the other batch
covers the latency.
"""

import numpy as np

T_GRID = 2048
B = 16
N = 2048          # Nc == Nt == 2048
NCORES = 8
BLOC = B // NCORES
EPS = 1e-8

_PROG_CACHE = {}


def build_program():
    import concourse.bacc as bacc
    import concourse.tile as tile
    from concourse import mybir

    f32 = mybir.dt.float32
    f16 = mybir.dt.float16
    AF = mybir.ActivationFunctionType
    # Bacc (not raw Bass): its compile() splits multi-sem waits into event
    # semaphores / ldweights, which the TRN2 ISA requires (1 wait per inst).
    nc = bacc.Bacc(None, target_bir_lowering=False)

    T1h = nc.declare_dram_parameter("T1", [3, T_GRID], f32, isOutput=False)
    T2h = nc.declare_dram_parameter("T2", [3, T_GRID], f32, isOutput=False)
    X1h = nc.declare_dram_parameter("X1", [BLOC, 3, 2, N], f32, isOutput=False)
    X2h = nc.declare_dram_parameter("X2", [BLOC, 3, 2, N], f32, isOutput=False)
    PHIh = nc.declare_dram_parameter("PHI", [BLOC, 128, 32], f16, isOutput=False)
    TCh = nc.declare_dram_parameter("TCONV", [16, T_GRID], f32, isOutput=False)
    W1h = nc.declare_dram_parameter("W1", [2, 80], f16, isOutput=False)
    W2h = nc.declare_dram_parameter("W2", [16, 160], f16, isOutput=False)
    W3h = nc.declare_dram_parameter("W3", [32, 80], f16, isOutput=False)
    W4muh = nc.declare_dram_parameter("W4mu", [16, 5], f16, isOutput=False)
    W4sgh = nc.declare_dram_parameter("W4sg", [16, 5], f16, isOutput=False)
    B2h = nc.declare_dram_parameter("B2", [32, 1], f32, isOutput=False)
    B3h = nc.declare_dram_parameter("B3", [16, 1], f32, isOutput=False)
    Ch = nc.declare_dram_parameter("CONSTS", [2, 4], f32, isOutput=False)
    ID2h = nc.declare_dram_parameter("ID2", [2, 2], f16, isOutput=False)
    OUTh = nc.declare_dram_parameter("out", [BLOC, 2, T_GRID], f32, isOutput=True)

    with tile.TileContext(nc) as tc:
        with (
            tc.tile_pool(name="singles", bufs=1) as singles,
            tc.tile_pool(name="perb", bufs=2) as perb,
            tc.tile_pool(name="kpool", bufs=3) as kpool,
            tc.tile_pool(name="small", bufs=1) as small,
            tc.tile_pool(name="outs", bufs=2) as outs,
            tc.tile_pool(name="psd2", bufs=2, space="PSUM") as psd2,
            tc.tile_pool(name="psacc", bufs=2, space="PSUM") as psacc,
        ):
            T1_sb = singles.tile([3, T_GRID], f32)
            nc.sync.dma_start(out=T1_sb, in_=T1h[:, :])
            T2_sb = singles.tile([3, T_GRID], f32)
            nc.sync.dma_start(out=T2_sb, in_=T2h[:, :])
            TC_sb = singles.tile([16, T_GRID], f32)
            nc.sync.dma_start(out=TC_sb, in_=TCh[:, :])
            W1_sb = singles.tile([2, 80], f16)
            nc.sync.dma_start(out=W1_sb, in_=W1h[:, :])
            W2_sb = singles.tile([16, 160], f16)
            nc.sync.dma_start(out=W2_sb, in_=W2h[:, :])
            W3_sb = singles.tile([32, 80], f16)
            nc.sync.dma_start(out=W3_sb, in_=W3h[:, :])
            W4mu_sb = singles.tile([16, 5], f16)
            nc.sync.dma_start(out=W4mu_sb, in_=W4muh[:, :])
            W4sg_sb = singles.tile([16, 5], f16)
            nc.sync.dma_start(out=W4sg_sb, in_=W4sgh[:, :])
            B2_sb = singles.tile([32, 1], f32)
            nc.sync.dma_start(out=B2_sb, in_=B2h[:, :])
            B3_sb = singles.tile([16, 1], f32)
            nc.sync.dma_start(out=B3_sb, in_=B3h[:, :])
            C_sb = singles.tile([2, 4], f32)
            nc.sync.dma_start(out=C_sb, in_=Ch[:, :])
            ID2_sb = singles.tile([2, 2], f16)
            nc.sync.dma_start(out=ID2_sb, in_=ID2h[:, :])

            st = [dict() for _ in range(BLOC)]  # per-batch tile handles

            def loads(b):
                s = st[b]
                s["X1"] = perb.tile([3, 2, N], f32, tag="X1", name="X1_sb")
                nc.sync.dma_start(out=s["X1"], in_=X1h[b])
                s["X2"] = perb.tile([3, 2, N], f32, tag="X2", name="X2_sb")
                nc.sync.dma_start(out=s["X2"], in_=X2h[b])
                s["PHI"] = perb.tile([128, 32], f16, tag="PHI", name="PHI_sb")
                nc.sync.dma_start(out=s["PHI"], in_=PHIh[b])
                rep2 = perb.tile([2, T_GRID + 4], f16, tag="rep2", name="rep2")
                nc.vector.memset(rep2[:, 0:2], 0.0)
                nc.vector.memset(rep2[:, T_GRID + 2 : T_GRID + 4], 0.0)
                s["rep2"] = rep2

            def stage_a(b):
                s = st[b]
                X1_sb, PHI_sb, rep2 = s["X1"], s["PHI"], s["rep2"]
                h_ps = [None, None]
                kq = []

                def gen_enc(sq):
                    n2, i = divmod(sq, 16)
                    K1 = kpool.tile([128, 1024], f16, tag="K", name="K1")
                    d2 = psd2.tile([128, 1024], f32, tag="d2", name="d2")
                    for hh in range(2):
                        nc.tensor.matmul(
                            d2[:, 512 * hh : 512 * (hh + 1)],
                            X1_sb[:, n2, 128 * i : 128 * (i + 1)],
                            T1_sb[:, 1024 * n2 + 512 * hh : 1024 * n2 + 512 * (hh + 1)],
                            start=True,
                            stop=True,
                        )
                    nc.scalar.activation(out=K1, in_=d2, func=AF.Exp)
                    kq.append((K1, n2, i))

                def acc_enc():
                    K1, n2, i = kq.pop(0)
                    if i == 0:
                        h_ps[n2] = psacc.tile([2, 1024], f32, tag="acc", name="h_acc")
                    for hh in range(2):
                        nc.tensor.matmul(
                            h_ps[n2][:, 512 * hh : 512 * (hh + 1)],
                            PHI_sb[:, 2 * i : 2 * i + 2],
                            K1[:, 512 * hh : 512 * (hh + 1)],
                            start=(i == 0),
                            stop=(i == 15),
                        )
                    if i == 15:
                        # single-row math in base-0 tiles; DMA (no partition
                        # base restriction) places rep2 rows 0/1.
                        sl = slice(2 + 1024 * n2, 2 + 1024 * (n2 + 1))
                        h_sb = small.tile([2, 1024], f32, tag="h_sb", name="h_sb")
                        h1_sb = small.tile([1, 1024], f32, tag="h1_sb", name="h1_sb")
                        tmp = small.tile([1, 1024], f32, tag="tmp", name="tmp")
                        rec = small.tile([1, 1024], f32, tag="rec", name="rec")
                        h0f = small.tile([1, 1024], f16, tag="h0f", name="h0f")
                        ratf = small.tile([1, 1024], f16, tag="ratf", name="ratf")
                        nc.vector.tensor_copy(h_sb, h_ps[n2][:, :])
                        nc.sync.dma_start(out=h1_sb, in_=h_sb[1:2, :])
                        nc.vector.tensor_scalar_add(tmp, h_sb[0:1, :], EPS)
                        nc.vector.reciprocal(rec, tmp)
                        nc.vector.tensor_copy(h0f, h_sb[0:1, :])
                        nc.vector.tensor_mul(ratf, h1_sb, rec)
                        nc.sync.dma_start(out=rep2[0:1, sl], in_=h0f)
                        nc.sync.dma_start(out=rep2[1:2, sl], in_=ratf)

                for sq in range(33):
                    if sq < 32:
                        gen_enc(sq)
                    if sq >= 1:
                        acc_enc()

            def stage_b_layer(b, l):
                """conv layer l for batch b: 5-tap fp16 accumulating matmuls
                over 512-wide chunks."""
                s = st[b]
                if l == 0:
                    for nmt, shp in (("f1", 16), ("f2", 32), ("f3", 16)):
                        s[nmt] = perb.tile([shp, T_GRID + 4], f16, tag=nmt, name=nmt)
                        nc.vector.memset(s[nmt][:, 0:2], 0.0)
                        nc.vector.memset(s[nmt][:, T_GRID + 2 : T_GRID + 4], 0.0)
                    s["fmu"] = perb.tile([1, T_GRID], f16, tag="fmu_r", name="fmu_r")
                    s["fsg"] = perb.tile([1, T_GRID], f16, tag="fsg_r", name="fsg_r")

                if l == 0:
                    # conv1: only h0/ratio rows on PE; the t-row term + b1 is
                    # the host-precomputed TCONV, added on DVE before relu.
                    for n in range(4):
                        ps = psacc.tile([16, 512], f32, tag="acc", name="c1ps")
                        for o in range(5):
                            nc.tensor.matmul(
                                ps,
                                W1_sb[:, o * 16 : (o + 1) * 16],
                                s["rep2"][:, 512 * n + o : 512 * n + o + 512],
                                start=(o == 0),
                                stop=(o == 4),
                            )
                        nc.vector.tensor_add(
                            ps, ps, TC_sb[:, 512 * n : 512 * (n + 1)]
                        )
                        nc.scalar.activation(
                            out=s["f1"][:, 2 + 512 * n : 2 + 512 * (n + 1)],
                            in_=ps,
                            func=AF.Relu,
                        )
                elif l in (1, 2):
                    in_tile, w_sb, bias_sb, O = (
                        (s["f1"], W2_sb, B2_sb, 32) if l == 1
                        else (s["f2"], W3_sb, B3_sb, 16)
                    )
                    out_tile = s["f2"] if l == 1 else s["f3"]
                    for n in range(4):
                        ps = psacc.tile([O, 512], f32, tag="acc", name="cps")
                        for o in range(5):
                            nc.tensor.matmul(
                                ps,
                                w_sb[:, o * O : (o + 1) * O],
                                in_tile[:, 512 * n + o : 512 * n + o + 512],
                                start=(o == 0),
                                stop=(o == 4),
                            )
                        nc.scalar.activation(
                            out=out_tile[:, 2 + 512 * n : 2 + 512 * (n + 1)],
                            in_=ps,
                            func=AF.Relu,
                            bias=bias_sb,
                        )
                else:
                    # conv4: mu and sigma rows accumulated separately so both
                    # sit at partition base 0.
                    for n in range(4):
                        ps_mu = psacc.tile([1, 512], f32, tag="acc", name="mu_ps")
                        ps_sg = psacc.tile([1, 512], f32, tag="acc", name="sg_ps")
                        for o in range(5):
                            rhs = s["f3"][:, 512 * n + o : 512 * n + o + 512]
                            nc.tensor.matmul(
                                ps_mu, W4mu_sb[:, o : o + 1], rhs,
                                start=(o == 0), stop=(o == 4),
                            )
                            nc.tensor.matmul(
                                ps_sg, W4sg_sb[:, o : o + 1], rhs,
                                start=(o == 0), stop=(o == 4),
                            )
                        sl = slice(512 * n, 512 * (n + 1))
                        sa = small.tile([1, 512], f32, tag="tmp", name="sa")
                        sr = small.tile([1, 512], f32, tag="rec", name="sr")
                        fsgt = small.tile([1, 512], f32, tag="h1_sb", name="fsgt")
                        nc.scalar.activation(
                            out=s["fmu"][0:1, sl],
                            in_=ps_mu,
                            func=AF.Identity,
                            bias=C_sb[0:1, 0:1],
                            scale=C_sb[0:1, 2:3],
                        )
                        # softplus(x+b) = relu(x+b) + ln(1 + exp(-|x+b|));
                        # no act table set has both Exp and Softplus, so
                        # compose from set-6 funcs (abs/exp/ln/relu).
                        nc.scalar.activation(
                            out=sa, in_=ps_sg, func=AF.Abs, bias=C_sb[0:1, 1:2]
                        )
                        nc.scalar.activation(out=sa, in_=sa, func=AF.Exp, scale=-1.0)
                        nc.scalar.activation(out=sa, in_=sa, func=AF.Ln, bias=1.0)
                        nc.scalar.activation(
                            out=sr, in_=ps_sg, func=AF.Relu, bias=C_sb[0:1, 1:2]
                        )
                        nc.vector.tensor_add(fsgt, sa, sr)
                        nc.vector.tensor_scalar_mul(
                            s["fsg"][0:1, sl], fsgt, C_sb[0:1, 2:3]
                        )

            def stage_t(b):
                # transpose fmu/fsg rows -> fT[p, c, j] = f'_c[128j+p]
                s = st[b]
                fT = perb.tile([128, 2, 16], f16, tag="fT", name="fT")
                s["fT"] = fT
                for j in range(16):
                    for c, row in enumerate((s["fmu"], s["fsg"])):
                        tp = psd2.tile([128, 1], f16, tag="d2", name="tp")
                        nc.tensor.transpose(
                            tp, row[0:1, 128 * j : 128 * (j + 1)], ID2_sb[0:1, 0:1]
                        )
                        nc.scalar.copy(fT[:, c : c + 1, j], tp)

            def stage_c(b):
                s = st[b]
                X2_sb, fT = s["X2"], s["fT"]
                ms_ps = [None, None]
                kq2 = []

                def gen_dec(sq):
                    n2, j = divmod(sq, 16)
                    K2 = kpool.tile([128, 1024], f16, tag="K", name="K2")
                    d2 = psd2.tile([128, 1024], f32, tag="d2", name="d2c")
                    for hh in range(2):
                        nc.tensor.matmul(
                            d2[:, 512 * hh : 512 * (hh + 1)],
                            T2_sb[:, 128 * j : 128 * (j + 1)],
                            X2_sb[:, j // 8,
                                  1024 * n2 + 512 * hh : 1024 * n2 + 512 * (hh + 1)],
                            start=True,
                            stop=True,
                        )
                    nc.scalar.activation(out=K2, in_=d2, func=AF.Exp)
                    kq2.append((K2, n2, j))

                def acc_dec():
                    K2, n2, j = kq2.pop(0)
                    if j == 0:
                        ms_ps[n2] = psacc.tile([2, 1024], f32, tag="acc", name="ms_acc")
                    for hh in range(2):
                        nc.tensor.matmul(
                            ms_ps[n2][:, 512 * hh : 512 * (hh + 1)],
                            fT[:, :, j],
                            K2[:, 512 * hh : 512 * (hh + 1)],
                            start=(j == 0),
                            stop=(j == 15),
                        )
                    if j == 15:
                        ms_sb = outs.tile([2, 1024], f32, tag="ms_sb", name="ms_sb")
                        nc.vector.tensor_copy(ms_sb, ms_ps[n2][:, :])
                        nc.sync.dma_start(
                            out=OUTh[b, :, 1024 * n2 : 1024 * (n2 + 1)],
                            in_=ms_sb,
                        )

                for sq in range(33):
                    if sq < 32:
                        gen_dec(sq)
                    if sq >= 1:
                        acc_dec()

            loads(0)
            loads(1)
            stage_a(0)
            stage_a(1)
            for l in range(4):
                for b in range(BLOC):
                    stage_b_layer(b, l)
            stage_t(0)
            stage_c(0)
            stage_t(1)
            stage_c(1)

    nc.compile()
    return nc


def make_inmaps(inputs):
    """Host-side table construction. Returns list of 8 per-core input dicts."""
    f32 = np.float32
    f16 = np.float16
    f64 = np.float64
    xc = np.asarray(inputs["xc"])[..., 0].astype(f32)
    yc = np.asarray(inputs["yc"])[..., 0].astype(f32)
    xt = np.asarray(inputs["xt"])[..., 0].astype(f32)
    ls_psi = f64(np.float32(inputs["ls_psi"]))
    os_psi = f64(np.float32(inputs["os_psi"]))
    ls_rho = f64(np.float32(inputs["ls_rho"]))
    os_rho = f64(np.float32(inputs["os_rho"]))
    w = [np.asarray(inputs[f"w{i}"]).astype(f32) for i in (1, 2, 3, 4)]
    bs = [np.asarray(inputs[f"b{i}"]).astype(f32) for i in (1, 2, 3, 4)]

    lower = np.minimum(xc.min(), xt.min())
    upper = np.maximum(xc.max(), xt.max())
    t64 = np.linspace(f64(lower), f64(upper), T_GRID)
    t = t64.astype(f32)

    a_psi = -0.5 / (ls_psi * ls_psi)
    a_rho = -0.5 / (ls_rho * ls_rho)

    cA = np.array([(t64[h * 1024] + t64[h * 1024 + 1023]) / 2 for h in range(2)])
    cB = np.array([(t64[j * 1024] + t64[j * 1024 + 1023]) / 2 for j in range(2)])

    T1 = np.zeros((3, T_GRID), f32)
    T2 = np.zeros((3, T_GRID), f32)
    for h in range(2):
        sl = slice(h * 1024, (h + 1) * 1024)
        tp = t64[sl] - cA[h]
        T1[0, sl] = 1.0
        T1[1, sl] = tp.astype(f32)
        T1[2, sl] = (tp * tp).astype(f32)
    for j in range(2):
        sl = slice(j * 1024, (j + 1) * 1024)
        tp = t64[sl] - cB[j]
        T2[0, sl] = (a_rho * tp * tp).astype(f32)
        T2[1, sl] = (-2.0 * a_rho * tp).astype(f32)
        T2[2, sl] = a_rho

    X1 = np.zeros((B, 3, 2, N), f32)
    X2 = np.zeros((B, 3, 2, N), f32)
    PHI = np.zeros((B, 128, 32), f32)
    for bi in range(B):
        xcb = xc[bi].astype(f64)
        xtb = xt[bi].astype(f64)
        for h in range(2):
            xp = xcb - cA[h]
            X1[bi, 0, h] = (a_psi * xp * xp).astype(f32)
            X1[bi, 1, h] = (-2.0 * a_psi * xp).astype(f32)
            X1[bi, 2, h] = a_psi
        for j in range(2):
            xp = xtb - cB[j]
            X2[bi, 0, j] = 1.0
            X2[bi, 1, j] = xp.astype(f32)
            X2[bi, 2, j] = (xp * xp).astype(f32)
        phi_full = np.stack([np.full(N, os_psi), os_psi * yc[bi].astype(f64)], 1)
        PHI[bi] = phi_full.astype(f32).reshape(16, 128, 2).transpose(1, 0, 2).reshape(128, 32)

    # TCONV[o, t] = sum_o' w1[o, 0, o'] * t_pad[t + o'] + b1[o]  (exact fp64)
    t_pad = np.zeros(T_GRID + 4, f64)
    t_pad[2 : 2 + T_GRID] = t64
    TCONV = np.zeros((16, T_GRID), f64)
    for o in range(5):
        TCONV += w[0][:, 0, o].astype(f64)[:, None] * t_pad[o : o + T_GRID][None, :]
    TCONV += bs[0].astype(f64)[:, None]

    def pack_taps(wl, rows=None):
        # [I', 5*O]: cols o*O:(o+1)*O = wl[:, rows, o].T
        O, I, _ = wl.shape
        r = slice(None) if rows is None else rows
        blocks = [wl[:, r, o].T for o in range(5)]
        return np.concatenate(blocks, 1).astype(f16)

    consts = np.zeros((2, 4), f32)
    consts[:, 0] = f32(os_rho * f64(bs[3][0]))
    consts[:, 1] = bs[3][1]
    consts[:, 2] = f32(os_rho)

    shared = {
        "T1": T1,
        "T2": T2,
        "TCONV": TCONV.astype(f32),
        "W1": pack_taps(w[0], rows=slice(1, 3)),          # [2, 80]
        "W2": pack_taps(w[1]),                            # [16, 160]
        "W3": pack_taps(w[2]),                            # [32, 80]
        "W4mu": np.stack([w[3][0, :, o] for o in range(5)], 1).astype(f16),  # [16,5]
        "W4sg": np.stack([w[3][1, :, o] for o in range(5)], 1).astype(f16),  # [16,5]
        "B2": bs[1][:, None].copy(),
        "B3": bs[2][:, None].copy(),
        "CONSTS": consts,
        "ID2": np.eye(2, dtype=f16),
    }
    in_maps = []
    for c in range(NCORES):
        sl = slice(c * BLOC, (c + 1) * BLOC)
        m = dict(shared)
        m["X1"] = np.ascontiguousarray(X1[sl])
        m["X2"] = np.ascontiguousarray(X2[sl])
        m["PHI"] = np.ascontiguousarray(PHI[sl].astype(f16))
        in_maps.append(m)
    return in_maps


def _get_program():
    if "nc" not in _PROG_CACHE:
        _PROG_CACHE["nc"] = build_program()
    return _PROG_CACHE["nc"]


def kernel(**inputs):
    from concourse.bass_utils import run_bass_kernel_spmd

    nc = _get_program()
    in_maps = make_inmaps(inputs)
    res = run_bass_kernel_spmd(nc, in_maps, core_ids=list(range(NCORES)))
    outs = [np.asarray(res.results[i]["out"]) for i in range(NCORES)]
    full = np.concatenate(outs, 0)  # [B, 2, T]
    return np.ascontiguousarray(full.transpose(0, 2, 1)).astype(np.float32)


# revision 35
# speedup vs baseline: 2.3737x; 2.0678x over previous
"""ConvCNP1d Trainium2 kernel.

Data-parallel over batch: 16 batches -> 8 cores x 2 batches. Each core:
  K1[x,t] = exp(a_psi*(x-t)^2) generated on PE as a rank-3 fp32 matmul
    (host-precomputed [a*x'^2, -2a*x', a] x [1, t', t'^2] tables, coords
    centered per t-chunk so the exponent partials stay small wherever the
    kernel value is non-negligible), Exp on the scalar engine into fp16,
  h = phi^T @ K1 accumulated on PE in fp16 (phi = os_psi*[1, yc] weights),
  conv1d stack as 5-tap fp16 accumulating matmuls; conv1's t-row
    contribution (linear, data-independent of the device) is precomputed
    on the host in fp64 and added via DVE, so only the h0 / h1-ratio rows
    run on-device; conv4 accumulates mu and sigma rows separately so both
    land at partition base 0 (compute engines can only address partition
    bases 0/32/64),
  K2[t,xt] generated the same way, mu/sigma = f'^T @ K2 in fp16.

Stages are emitted interleaved across the two batches
(A0 A1 B0 B1 T0 C0 T1 C1) so the PE always has matmul work while the
h-epilogue chains (DVE + two SBUF-to-SBUF DMA row moves) drain.
SBUF-to-SBUF DMAs have ~15us latency on this path, so the kernel uses
them only in the h epilogue, where stage-A work of the other batch
covers the latency.
"""

import numpy as np

T_GRID = 2048
B = 16
N = 2048          # Nc == Nt == 2048
NCORES = 8
BLOC = B // NCORES
EPS = 1e-8

_PROG_CACHE = {}


def build_program():
    import concourse.bacc as bacc
    import concourse.tile as tile
    from concourse import mybir

    f32 = mybir.dt.float32
    f16 = mybir.dt.float16
    AF = mybir.ActivationFunctionType
    # Bacc (not raw Bass): its compile() splits multi-sem waits into event
    # semaphores / ldweights, which the TRN2 ISA requires (1 wait per inst).
    nc = bacc.Bacc(None, target_bir_lowering=False)

    TPh = nc.declare_dram_parameter("TP_BC", [1, T_GRID], f32, isOutput=False)
    TSQh = nc.declare_dram_parameter("TSQ_BC", [1, T_GRID], f32, isOutput=False)
    XTh = nc.declare_dram_parameter("XT_BC", [BLOC, 1, T_GRID], f32, isOutput=False)
    XS1h = nc.declare_dram_parameter("XS1", [BLOC, 128, 2, 16], f32, isOutput=False)
    XB1h = nc.declare_dram_parameter("XB1", [BLOC, 128, 2, 16], f32, isOutput=False)
    TCPh = nc.declare_dram_parameter("TCP", [128, 16], f32, isOutput=False)
    AVh = nc.declare_dram_parameter("AVEC", [128, 2], f32, isOutput=False)
    PHIh = nc.declare_dram_parameter("PHI", [BLOC, 128, 32], f16, isOutput=False)
    TCh = nc.declare_dram_parameter("TCONV", [16, T_GRID], f32, isOutput=False)
    W1h = nc.declare_dram_parameter("W1", [2, 80], f16, isOutput=False)
    W2h = nc.declare_dram_parameter("W2", [16, 160], f16, isOutput=False)
    W3h = nc.declare_dram_parameter("W3", [32, 80], f16, isOutput=False)
    W4muh = nc.declare_dram_parameter("W4mu", [16, 5], f16, isOutput=False)
    W4sgh = nc.declare_dram_parameter("W4sg", [16, 5], f16, isOutput=False)
    B2h = nc.declare_dram_parameter("B2", [32, 1], f32, isOutput=False)
    B3h = nc.declare_dram_parameter("B3", [16, 1], f32, isOutput=False)
    Ch = nc.declare_dram_parameter("CONSTS", [2, 4], f32, isOutput=False)
    ID2h = nc.declare_dram_parameter("ID2", [2, 2], f16, isOutput=False)
    OUTh = nc.declare_dram_parameter("out", [BLOC, 2, T_GRID], f32, isOutput=True)

    with tile.TileContext(nc) as tc:
        with (
            tc.tile_pool(name="singles", bufs=1) as singles,
            tc.tile_pool(name="perb", bufs=2) as perb,
            tc.tile_pool(name="kpool", bufs=3) as kpool,
            tc.tile_pool(name="small", bufs=1) as small,
            tc.tile_pool(name="outs", bufs=2) as outs,
            tc.tile_pool(name="dvp", bufs=3) as dvp,
            tc.tile_pool(name="psd2", bufs=2, space="PSUM") as psd2,
            tc.tile_pool(name="psacc", bufs=3, space="PSUM") as psacc,
        ):
            import concourse.bass as bass_mod

            def bcast128(dst, src_ap):
                bc = bass_mod.AP(
                    tensor=src_ap.tensor, offset=src_ap.offset,
                    ap=[[0, 128], [1, T_GRID]],
                )
                nc.sync.dma_start(out=dst, in_=bc)

            TP_sb = singles.tile([128, T_GRID], f32)
            bcast128(TP_sb, TPh[:, :])
            TSQ_sb = singles.tile([128, T_GRID], f32)
            bcast128(TSQ_sb, TSQh[:, :])
            TCP_sb = singles.tile([128, 16], f32)
            nc.sync.dma_start(out=TCP_sb, in_=TCPh[:, :])
            AV_sb = singles.tile([128, 2], f32)
            nc.sync.dma_start(out=AV_sb, in_=AVh[:, :])
            TC_sb = singles.tile([16, T_GRID], f32)
            nc.sync.dma_start(out=TC_sb, in_=TCh[:, :])
            W1_sb = singles.tile([2, 80], f16)
            nc.sync.dma_start(out=W1_sb, in_=W1h[:, :])
            W2_sb = singles.tile([16, 160], f16)
            nc.sync.dma_start(out=W2_sb, in_=W2h[:, :])
            W3_sb = singles.tile([32, 80], f16)
            nc.sync.dma_start(out=W3_sb, in_=W3h[:, :])
            W4mu_sb = singles.tile([16, 5], f16)
            nc.sync.dma_start(out=W4mu_sb, in_=W4muh[:, :])
            W4sg_sb = singles.tile([16, 5], f16)
            nc.sync.dma_start(out=W4sg_sb, in_=W4sgh[:, :])
            B2_sb = singles.tile([32, 1], f32)
            nc.sync.dma_start(out=B2_sb, in_=B2h[:, :])
            B3_sb = singles.tile([16, 1], f32)
            nc.sync.dma_start(out=B3_sb, in_=B3h[:, :])
            C_sb = singles.tile([2, 4], f32)
            nc.sync.dma_start(out=C_sb, in_=Ch[:, :])
            ID2_sb = singles.tile([2, 2], f16)
            nc.sync.dma_start(out=ID2_sb, in_=ID2h[:, :])

            st = [dict() for _ in range(BLOC)]  # per-batch tile handles

            def loads(b):
                s = st[b]
                s["XS1"] = perb.tile([128, 2, 16], f32, tag="XS1", name="XS1_sb")
                nc.sync.dma_start(out=s["XS1"], in_=XS1h[b])
                s["XB1"] = perb.tile([128, 2, 16], f32, tag="XB1", name="XB1_sb")
                nc.sync.dma_start(out=s["XB1"], in_=XB1h[b])
                xt_rep = perb.tile([128, T_GRID], f32, tag="xt_rep", name="xt_rep")
                xsrc = XTh[b]
                bc = bass_mod.AP(
                    tensor=xsrc.tensor, offset=xsrc.offset,
                    ap=[[0, 128], [1, T_GRID]],
                )
                nc.sync.dma_start(out=xt_rep, in_=bc)
                s["xt_rep"] = xt_rep
                s["PHI"] = perb.tile([128, 32], f16, tag="PHI", name="PHI_sb")
                nc.sync.dma_start(out=s["PHI"], in_=PHIh[b])
                rep2 = perb.tile([2, T_GRID + 4], f16, tag="rep2", name="rep2")
                nc.vector.memset(rep2[:, 0:2], 0.0)
                nc.vector.memset(rep2[:, T_GRID + 2 : T_GRID + 4], 0.0)
                s["rep2"] = rep2

            def stage_a(b):
                s = st[b]
                XS1_sb, XB1_sb, PHI_sb, rep2 = (
                    s["XS1"], s["XB1"], s["PHI"], s["rep2"]
                )
                h_ps = [None, None]
                kq = []

                def gen_enc(sq):
                    # d2 = t'^2 - 2x'*t' in one fused DVE/GpSimd op; the
                    # a*x'^2 term rides in as the exp bias.
                    n2, i = divmod(sq, 16)
                    eng = nc.vector
                    sl = slice(1024 * n2, 1024 * (n2 + 1))
                    d2s = dvp.tile([128, 1024], f32, tag="d2s", name="d2s")
                    eng.scalar_tensor_tensor(
                        d2s,
                        TP_sb[:, sl],
                        XS1_sb[:, n2, i : i + 1],
                        TSQ_sb[:, sl],
                        mybir.AluOpType.mult,
                        mybir.AluOpType.add,
                    )
                    K1 = kpool.tile([128, 1024], f16, tag="K", name="K1")
                    nc.scalar.activation(
                        out=K1, in_=d2s, func=AF.Exp,
                        scale=AV_sb[:, 0:1], bias=XB1_sb[:, n2, i : i + 1],
                    )
                    kq.append((K1, n2, i))

                def acc_enc():
                    K1, n2, i = kq.pop(0)
                    if i == 0:
                        h_ps[n2] = psacc.tile([2, 1024], f32, tag="acc", name="h_acc")
                    for hh in range(2):
                        nc.tensor.matmul(
                            h_ps[n2][:, 512 * hh : 512 * (hh + 1)],
                            PHI_sb[:, 2 * i : 2 * i + 2],
                            K1[:, 512 * hh : 512 * (hh + 1)],
                            start=(i == 0),
                            stop=(i == 15),
                        )
                    if i == 15:
                        # single-row math in base-0 tiles; DMA (no partition
                        # base restriction) places rep2 rows 0/1.
                        sl = slice(2 + 1024 * n2, 2 + 1024 * (n2 + 1))
                        h_sb = small.tile([2, 1024], f32, tag="h_sb", name="h_sb")
                        h1_sb = small.tile([1, 1024], f32, tag="h1_sb", name="h1_sb")
                        tmp = small.tile([1, 1024], f32, tag="tmp", name="tmp")
                        rec = small.tile([1, 1024], f32, tag="rec", name="rec")
                        h0f = small.tile([1, 1024], f16, tag="h0f", name="h0f")
                        ratf = small.tile([1, 1024], f16, tag="ratf", name="ratf")
                        nc.vector.tensor_copy(h_sb, h_ps[n2][:, :])
                        nc.sync.dma_start(out=h1_sb, in_=h_sb[1:2, :])
                        nc.vector.tensor_scalar_add(tmp, h_sb[0:1, :], EPS)
                        nc.vector.reciprocal(rec, tmp)
                        nc.vector.tensor_copy(h0f, h_sb[0:1, :])
                        nc.vector.tensor_mul(ratf, h1_sb, rec)
                        nc.sync.dma_start(out=rep2[0:1, sl], in_=h0f)
                        nc.sync.dma_start(out=rep2[1:2, sl], in_=ratf)

                for sq in range(33):
                    if sq < 32:
                        gen_enc(sq)
                    if sq >= 1:
                        acc_enc()

            def stage_b_layer(b, l):
                """conv layer l for batch b: 5-tap fp16 accumulating matmuls
                over 512-wide chunks."""
                s = st[b]
                if l == 0:
                    for nmt, shp in (("f1", 16), ("f2", 32), ("f3", 16)):
                        s[nmt] = perb.tile([shp, T_GRID + 4], f16, tag=nmt, name=nmt)
                        nc.vector.memset(s[nmt][:, 0:2], 0.0)
                        nc.vector.memset(s[nmt][:, T_GRID + 2 : T_GRID + 4], 0.0)
                    s["fmu"] = perb.tile([1, T_GRID], f16, tag="fmu_r", name="fmu_r")
                    s["fsg"] = perb.tile([1, T_GRID], f16, tag="fsg_r", name="fsg_r")

                if l == 0:
                    # conv1: only h0/ratio rows on PE; the t-row term + b1 is
                    # the host-precomputed TCONV, added on DVE before relu.
                    for n in range(4):
                        ps = psacc.tile([16, 512], f32, tag="acc", name="c1ps")
                        for o in range(5):
                            nc.tensor.matmul(
                                ps,
                                W1_sb[:, o * 16 : (o + 1) * 16],
                                s["rep2"][:, 512 * n + o : 512 * n + o + 512],
                                start=(o == 0),
                                stop=(o == 4),
                            )
                        nc.vector.tensor_add(
                            ps, ps, TC_sb[:, 512 * n : 512 * (n + 1)]
                        )
                        nc.scalar.activation(
                            out=s["f1"][:, 2 + 512 * n : 2 + 512 * (n + 1)],
                            in_=ps,
                            func=AF.Relu,
                        )
                elif l in (1, 2):
                    in_tile, w_sb, bias_sb, O = (
                        (s["f1"], W2_sb, B2_sb, 32) if l == 1
                        else (s["f2"], W3_sb, B3_sb, 16)
                    )
                    out_tile = s["f2"] if l == 1 else s["f3"]
                    for n in range(4):
                        ps = psacc.tile([O, 512], f32, tag="acc", name="cps")
                        for o in range(5):
                            nc.tensor.matmul(
                                ps,
                                w_sb[:, o * O : (o + 1) * O],
                                in_tile[:, 512 * n + o : 512 * n + o + 512],
                                start=(o == 0),
                                stop=(o == 4),
                            )
                        nc.scalar.activation(
                            out=out_tile[:, 2 + 512 * n : 2 + 512 * (n + 1)],
                            in_=ps,
                            func=AF.Relu,
                            bias=bias_sb,
                        )
                else:
                    # conv4: mu and sigma rows accumulated separately so both
                    # sit at partition base 0.
                    for n in range(4):
                        ps_mu = psacc.tile([1, 512], f32, tag="acc", name="mu_ps")
                        ps_sg = psacc.tile([1, 512], f32, tag="acc", name="sg_ps")
                        for o in range(5):
                            rhs = s["f3"][:, 512 * n + o : 512 * n + o + 512]
                            nc.tensor.matmul(
                                ps_mu, W4mu_sb[:, o : o + 1], rhs,
                                start=(o == 0), stop=(o == 4),
                            )
                            nc.tensor.matmul(
                                ps_sg, W4sg_sb[:, o : o + 1], rhs,
                                start=(o == 0), stop=(o == 4),
                            )
                        sl = slice(512 * n, 512 * (n + 1))
                        sa = small.tile([1, 512], f32, tag="tmp", name="sa")
                        sr = small.tile([1, 512], f32, tag="rec", name="sr")
                        fsgt = small.tile([1, 512], f32, tag="h1_sb", name="fsgt")
                        nc.scalar.activation(
                            out=s["fmu"][0:1, sl],
                            in_=ps_mu,
                            func=AF.Identity,
                            bias=C_sb[0:1, 0:1],
                            scale=C_sb[0:1, 2:3],
                        )
                        # softplus(x+b) = relu(x+b) + ln(1 + exp(-|x+b|));
                        # no act table set has both Exp and Softplus, so
                        # compose from set-6 funcs (abs/exp/ln/relu).
                        nc.scalar.activation(
                            out=sa, in_=ps_sg, func=AF.Abs, bias=C_sb[0:1, 1:2]
                        )
                        nc.scalar.activation(out=sa, in_=sa, func=AF.Exp, scale=-1.0)
                        nc.scalar.activation(out=sa, in_=sa, func=AF.Ln, bias=1.0)
                        nc.scalar.activation(
                            out=sr, in_=ps_sg, func=AF.Relu, bias=C_sb[0:1, 1:2]
                        )
                        nc.vector.tensor_add(fsgt, sa, sr)
                        nc.vector.tensor_scalar_mul(
                            s["fsg"][0:1, sl], fsgt, C_sb[0:1, 2:3]
                        )

            def stage_t(b):
                # transpose fmu/fsg rows -> fT[p, c, j] = f'_c[128j+p]
                s = st[b]
                fT = perb.tile([128, 2, 16], f16, tag="fT", name="fT")
                s["fT"] = fT
                for j in range(16):
                    for c, row in enumerate((s["fmu"], s["fsg"])):
                        tp = psd2.tile([128, 1], f16, tag="d2", name="tp")
                        nc.tensor.transpose(
                            tp, row[0:1, 128 * j : 128 * (j + 1)], ID2_sb[0:1, 0:1]
                        )
                        nc.scalar.copy(fT[:, c : c + 1, j], tp)

            def stage_c(b):
                s = st[b]
                xt_rep, fT = s["xt_rep"], s["fT"]
                ms_ps = [None, None]
                kq2 = []

                def gen_dec(sq):
                    # d2 = (xt - t_p)^2 computed exactly: sub then square.
                    n2, j = divmod(sq, 16)
                    eng = nc.vector
                    sl = slice(1024 * n2, 1024 * (n2 + 1))
                    dsub = dvp.tile([128, 1024], f32, tag="dsub", name="dsub")
                    eng.tensor_scalar_sub(
                        dsub, xt_rep[:, sl], TCP_sb[:, j : j + 1]
                    )
                    d2s = dvp.tile([128, 1024], f32, tag="d2s", name="d2c")
                    eng.tensor_mul(d2s, dsub, dsub)
                    K2 = kpool.tile([128, 1024], f16, tag="K", name="K2")
                    nc.scalar.activation(
                        out=K2, in_=d2s, func=AF.Exp, scale=AV_sb[:, 1:2]
                    )
                    kq2.append((K2, n2, j))

                def acc_dec():
                    K2, n2, j = kq2.pop(0)
                    if j == 0:
                        ms_ps[n2] = psacc.tile([2, 1024], f32, tag="acc", name="ms_acc")
                    for hh in range(2):
                        nc.tensor.matmul(
                            ms_ps[n2][:, 512 * hh : 512 * (hh + 1)],
                            fT[:, :, j],
                            K2[:, 512 * hh : 512 * (hh + 1)],
                            start=(j == 0),
                            stop=(j == 15),
                        )
                    if j == 15:
                        ms_sb = outs.tile([2, 1024], f32, tag="ms_sb", name="ms_sb")
                        nc.vector.tensor_copy(ms_sb, ms_ps[n2][:, :])
                        nc.sync.dma_start(
                            out=OUTh[b, :, 1024 * n2 : 1024 * (n2 + 1)],
                            in_=ms_sb,
                        )

                for sq in range(33):
                    if sq < 32:
                        gen_dec(sq)
                    if sq >= 1:
                        acc_dec()

            loads(0)
            loads(1)
            stage_a(0)
            stage_a(1)
            for l in range(4):
                for b in range(BLOC):
                    stage_b_layer(b, l)
            stage_t(0)
            stage_c(0)
            stage_t(1)
            stage_c(1)

    nc.compile()
    return nc


def make_inmaps(inputs):
    """Host-side table construction. Returns list of 8 per-core input dicts."""
    f32 = np.float32
    f16 = np.float16
    f64 = np.float64
    xc = np.asarray(inputs["xc"])[..., 0].astype(f32)
    yc = np.asarray(inputs["yc"])[..., 0].astype(f32)
    xt = np.asarray(inputs["xt"])[..., 0].astype(f32)
    ls_psi = f64(np.float32(inputs["ls_psi"]))
    os_psi = f64(np.float32(inputs["os_psi"]))
    ls_rho = f64(np.float32(inputs["ls_rho"]))
    os_rho = f64(np.float32(inputs["os_rho"]))
    w = [np.asarray(inputs[f"w{i}"]).astype(f32) for i in (1, 2, 3, 4)]
    bs = [np.asarray(inputs[f"b{i}"]).astype(f32) for i in (1, 2, 3, 4)]

    lower = np.minimum(xc.min(), xt.min())
    upper = np.maximum(xc.max(), xt.max())
    t64 = np.linspace(f64(lower), f64(upper), T_GRID)
    t = t64.astype(f32)

    a_psi = -0.5 / (ls_psi * ls_psi)
    a_rho = -0.5 / (ls_rho * ls_rho)

    cA = np.array([(t64[h * 1024] + t64[h * 1024 + 1023]) / 2 for h in range(2)])

    # t' tables (centered per 1024-half) for the fused encoder exponent
    TP = np.zeros((1, T_GRID), f32)
    TSQ = np.zeros((1, T_GRID), f32)
    for h in range(2):
        sl = slice(h * 1024, (h + 1) * 1024)
        tp = t64[sl] - cA[h]
        TP[0, sl] = tp.astype(f32)
        TSQ[0, sl] = (tp * tp).astype(f32)
    TCP = t.reshape(16, 128).T.copy()            # TCP[p, j] = t[128j + p]
    AVEC = np.zeros((128, 2), f32)
    AVEC[:, 0] = f32(a_psi)
    AVEC[:, 1] = f32(a_rho)

    XS1 = np.zeros((B, 128, 2, 16), f32)
    XB1 = np.zeros((B, 128, 2, 16), f32)
    PHI = np.zeros((B, 128, 32), f32)
    for bi in range(B):
        xcb = xc[bi].astype(f64).reshape(16, 128)   # [i, p]
        for h in range(2):
            xp = xcb - cA[h]
            XS1[bi, :, h, :] = (-2.0 * xp).astype(f32).T
            XB1[bi, :, h, :] = (a_psi * xp * xp).astype(f32).T
        phi_full = np.stack([np.full(N, os_psi), os_psi * yc[bi].astype(f64)], 1)
        PHI[bi] = phi_full.astype(f32).reshape(16, 128, 2).transpose(1, 0, 2).reshape(128, 32)

    # TCONV[o, t] = sum_o' w1[o, 0, o'] * t_pad[t + o'] + b1[o]  (exact fp64)
    t_pad = np.zeros(T_GRID + 4, f64)
    t_pad[2 : 2 + T_GRID] = t64
    TCONV = np.zeros((16, T_GRID), f64)
    for o in range(5):
        TCONV += w[0][:, 0, o].astype(f64)[:, None] * t_pad[o : o + T_GRID][None, :]
    TCONV += bs[0].astype(f64)[:, None]

    def pack_taps(wl, rows=None):
        # [I', 5*O]: cols o*O:(o+1)*O = wl[:, rows, o].T
        O, I, _ = wl.shape
        r = slice(None) if rows is None else rows
        blocks = [wl[:, r, o].T for o in range(5)]
        return np.concatenate(blocks, 1).astype(f16)

    consts = np.zeros((2, 4), f32)
    consts[:, 0] = f32(os_rho * f64(bs[3][0]))
    consts[:, 1] = bs[3][1]
    consts[:, 2] = f32(os_rho)

    shared = {
        "TP_BC": TP,
        "TSQ_BC": TSQ,
        "TCP": TCP,
        "AVEC": AVEC,
        "TCONV": TCONV.astype(f32),
        "W1": pack_taps(w[0], rows=slice(1, 3)),          # [2, 80]
        "W2": pack_taps(w[1]),                            # [16, 160]
        "W3": pack_taps(w[2]),                            # [32, 80]
        "W4mu": np.stack([w[3][0, :, o] for o in range(5)], 1).astype(f16),  # [16,5]
        "W4sg": np.stack([w[3][1, :, o] for o in range(5)], 1).astype(f16),  # [16,5]
        "B2": bs[1][:, None].copy(),
        "B3": bs[2][:, None].copy(),
        "CONSTS": consts,
        "ID2": np.eye(2, dtype=f16),
    }
    in_maps = []
    for c in range(NCORES):
        sl = slice(c * BLOC, (c + 1) * BLOC)
        m = dict(shared)
        m["XS1"] = np.ascontiguousarray(XS1[sl])
        m["XB1"] = np.ascontiguousarray(XB1[sl])
        m["XT_BC"] = np.ascontiguousarray(xt[sl][:, None, :])
        m["PHI"] = np.ascontiguousarray(PHI[sl].astype(f16))
        in_maps.append(m)
    return in_maps


def _get_program():
    if "nc" not in _PROG_CACHE:
        _PROG_CACHE["nc"] = build_program()
    return _PROG_CACHE["nc"]


def kernel(**inputs):
    from concourse.bass_utils import run_bass_kernel_spmd

    nc = _get_program()
    in_maps = make_inmaps(inputs)
    res = run_bass_kernel_spmd(nc, in_maps, core_ids=list(range(NCORES)))
    outs = [np.asarray(res.results[i]["out"]) for i in range(NCORES)]
    full = np.concatenate(outs, 0)  # [B, 2, T]
    return np.ascontiguousarray(full.transpose(0, 2, 1)).astype(np.float32)
